# revision 31
# baseline (speedup 1.0000x reference)
"""Distributed multi-head causal attention for 8 TRN2 NeuronCores.

Problem: B=4, T=2048, D=2048, H=16 heads of dk=dv=128.
  out = softmax(mask((q@Wq)(k@Wk)^T / sqrt(dk))) @ (v@Wv) @ Wo

Sharding (2D; all per-core asymmetry lives in host-supplied data so the
SPMD graph is identical on every core):
  core c -> batch b = c//2, head-group g = c%2 (heads 8g..8g+7).
  - QKV projections + attention for (batch b, its 8 heads): fully local.
  - Pair AllGather (replica groups [2b, 2b+1]) exchanges the per-head
    attention outputs (merged^T, bf16) per q-chunk.
  - Output projection: each core computes out^T for its batch for HALF
    the output columns (even core: cols 0..1023, odd: 1024..2047).
  Host reassembles: out[b] = concat(outT_2b, outT_2b+1, axis=0).T

Performance structure (v3):
  - All intermediates (Q^T/K^T per head, V natural) stay RESIDENT IN
    SBUF - no DRAM round trip, no attention-phase input DMAs.
  - Two HWDGE rings: sync (SP) carries weights wv01/wk/wq + cc staging
    + mfq; scalar (ACT) carries activation streams + wv23 + wo + outT.
  - Attention chunks run IN ORDER (0,1,2,3). The per-block s->exp->o
    latency bubbles of the in-order PE queue are absorbed by a FILLER
    stream of independent matmuls interleaved between attention
    matmuls: first the deferred Q3 projection (consumed during
    att0/att1), then the wo output projections of already-gathered
    chunks (wo0 from att2, wo1 during att3, the rest as a dense tail
    whose tail-end runway covers the chunk-3 gather latency).
  - wv is split across BOTH DMA rings at kernel start so the first
    V-projection PSUM accumulation (which needs all 16 k-slices of wv)
    never stalls; N_WARM dummy matmuls cover DMA-ring bootstrap and
    keep the PE HAM clock gate at K=8/8.

Compute is bf16 on TensorE with f32 PSUM accumulation. Softmax skips the
max-subtraction (scores are ~N(0,1); exp is safe in f32) and obtains the
denominators with a ones-matmul per 8 exp-blocks (DVE-accumulated bf16
partial sums); causal masking multiplies exp(scores) by a 0/1 triangular
tile on the single diagonal-crossing 128x128 sub-block, and the moving
free dim of diagonal-region matmuls is trimmed to the unmasked columns.
"""
import os
import sys
from contextlib import ExitStack

import numpy as np
import ml_dtypes

import concourse.bass as bass
import concourse.mybir as mybir
import concourse.tile as tile
from concourse import bacc
from concourse.bass_utils import run_bass_kernel_spmd

BF16 = mybir.dt.bfloat16
F32 = mybir.dt.float32

B, T, D = 4, 2048, 2048
H, DK, DV = 16, 128, 128
HG = 8                      # heads per core
N_CORES = 8
QC = 512                    # q-chunk (matmul moving free dim)
NQC = T // QC               # 4
NKB = T // 128              # 16 k-blocks
NDC = D // 128              # 16 contraction chunks
NCOL = D // 2 // 128        # 8 output-projection column blocks per core
SCALE = 1.0 / np.sqrt(DK)
N_WARM = 300                # dummy matmuls to warm the PE clock gate

_KERNEL_CACHE = {}


def build_kernel_causal():
    nc = bacc.Bacc("TRN2", num_devices=N_CORES)

    qT = nc.declare_dram_parameter("qT", [D, T], BF16, isOutput=False)
    kT = nc.declare_dram_parameter("kT", [D, T], BF16, isOutput=False)
    vT = nc.declare_dram_parameter("vT", [D, T], BF16, isOutput=False)
    wq = nc.declare_dram_parameter("wq", [D, HG * DK], BF16, isOutput=False)
    wk = nc.declare_dram_parameter("wk", [D, HG * DK], BF16, isOutput=False)
    wv = nc.declare_dram_parameter("wv", [D, HG * DV], BF16, isOutput=False)
    wo = nc.declare_dram_parameter("wo", [H * DV, D // 2], BF16, isOutput=False)
    tri = nc.declare_dram_parameter("tri", [128, 128], BF16, isOutput=False)
    outT = nc.declare_dram_parameter("outT", [D // 2, T], F32, isOutput=True)

    # Collective staging (collectives require DRAM in/out). Chunks 0-2:
    # ONE pair AllGather per q-chunk (in [128, HG, QC]) - their ~30us
    # fire-to-land latency is hidden by the schedule. Chunk 3 is split
    # into per-HEAD-PAIR gathers (4 x 256KB, fired after h1/h3/h5/h7):
    # the last one lands ~10us after the last head instead of ~33us,
    # which is what the wo3 tail actually waits on. (Fully per-head
    # gathers were tried and are CC-stream throughput-bound: ~6.7us
    # fixed stream occupancy per op x 32 ops backlogs the stream.)
    cc_in = {qc: nc.dram_tensor(f"cc_in_{qc}", [128, HG, QC], BF16)
             for qc in range(2)}
    cc_out = {qc: nc.dram_tensor(f"cc_out_{qc}", [2, 128, HG, QC], BF16)
              for qc in range(2)}
    cc2_in = {j: nc.dram_tensor(f"cc2_in_{j}", [128, 4, QC], BF16)
              for j in range(2)}
    cc2_out = {j: nc.dram_tensor(f"cc2_out_{j}", [2, 128, 4, QC], BF16)
               for j in range(2)}
    cc3_in = {j: nc.dram_tensor(f"cc3_in_{j}", [128, 2, QC], BF16)
              for j in range(4)}
    cc3_out = {j: nc.dram_tensor(f"cc3_out_{j}", [2, 128, 2, QC], BF16)
               for j in range(4)}
    cc_warm_in = nc.dram_tensor("cc_warm_in", [128, 8], BF16)
    cc_warm_out = nc.dram_tensor("cc_warm_out", [2, 128, 8], BF16)
    pair_groups = [[0, 1], [2, 3], [4, 5], [6, 7]]

    def kb_start(qc, kb):
        """First unmasked q column (within the chunk) for this k-block."""
        return min(max((kb - 4 * qc) * 128, 0), QC)

    with tile.TileContext(nc) as tc, ExitStack() as top:
        ent = top.enter_context
        # Pool releases must be LIFO (stack allocator), so creation
        # order is release-reverse: live-to-end pools first (consts,
        # res, B, the attention pools), then A (wv->wq, dies at
        # Q3-drain), then the phase-1 x streams (die at phase-1 end).
        # xq / mfq+ob are pushed later at the then-top of the stack.
        consts = ent(tc.tile_pool(name="consts", bufs=1))
        # SBUF-resident per-head projections (live for the whole kernel).
        res_pool = ent(tc.tile_pool(name="res", bufs=1))
        # Weight zones: A carries wv then (reused) wq; B carries wk then
        # (reused) wo. A closes mid-phase-2 (after the deferred Q3 proj
        # is done); B lives to the end.
        pool_B = ent(tc.tile_pool(name="wB", bufs=1))
        pt_pool = ent(tc.tile_pool(name="pt", bufs=4))
        racc_pool = ent(tc.tile_pool(name="racc", bufs=2))
        mstage = ent(tc.tile_pool(name="mstage", bufs=2))
        rinv_pool = ent(tc.tile_pool(name="rinv", bufs=2))
        stack_A = ExitStack()
        pool_A = stack_A.enter_context(tc.tile_pool(name="wA", bufs=1))
        # Phase-1-only: double-buffered activation streams.
        stack_X = ExitStack()
        x_pool = stack_X.enter_context(tc.tile_pool(name="xs", bufs=2))

        ones_sb = consts.tile([128, 128], BF16)
        nc.vector.memset(ones_sb, 1.0)
        tri_sb = consts.tile([128, 128], BF16)
        nc.sync.dma_start(out=tri_sb, in_=tri[:])

        # Warm the CC stream: the first collective of a NEFF pays the
        # stream barrier + cold-start (~10-30us extra). Fire it now so
        # that happens under the projection phase.
        nc.gpsimd.collective_compute(
            "AllGather",
            mybir.AluOpType.bypass,
            ins=[cc_warm_in[:]],
            outs=[cc_warm_out[:]],
            replica_groups=pair_groups,
        )

        q_all = res_pool.tile([128, HG, T], BF16, name="q_all")
        k_all = res_pool.tile([128, HG, T], BF16, name="k_all")
        v_all = res_pool.tile([128, HG, NKB, DV], BF16, name="v_all")

        def weight_tile(pool):
            return pool.tile([128, NDC, HG * 128], BF16, tag="w", name="wtile")

        def weight_slice(w_sb, w_ext, dci, eng=None):
            (eng or nc.sync).dma_start(
                out=w_sb[:, dci, :], in_=w_ext[dci * 128 : (dci + 1) * 128, :]
            )

        # wv is the startup-critical load: the first V-proj accumulation
        # needs all 16 k-slices, and every DMA ring takes ~10-15us to
        # bootstrap at kernel start. Spread the four 1MB chunks across
        # the THREE DGE rings (sync x2 / gpsimd / scalar-behind-xv0) so
        # they all land by ~20us, inside the warmup window.
        wv_sb = weight_tile(pool_A)
        for c, eng in ((0, nc.sync), (1, nc.sync), (2, nc.gpsimd),
                       (3, nc.sync)):
            eng.dma_start(
                out=wv_sb[:, 4 * c : 4 * (c + 1), :],
                in_=wv[4 * c * 128 : 4 * (c + 1) * 128, :].rearrange(
                    "(o p) f -> p o f", p=128
                ),
            )

        def x_stream(src, qc):
            """[128, NDC, QC] slice of an x^T input, contraction on
            partitions, via the ACT HWDGE ring."""
            xs = x_pool.tile([128, NDC, QC], BF16, tag="xs")
            nc.scalar.dma_start(
                out=xs,
                in_=src[:, qc * QC : (qc + 1) * QC].rearrange(
                    "(o p) f -> p o f", p=128
                ),
            )
            return xs

        xv0 = x_stream(vT, 0)
        wk_sb = weight_tile(pool_B)
        wq_sb = None  # allocated after V proj (reuses wv zone)

        # ------------- Phase 1: V, K, Q0-Q2 projections -------------
        with (
            tc.tile_pool(name="warmps", bufs=1, space="PSUM") as warmps,
            tc.tile_pool(name="ppsum", bufs=4, space="PSUM") as ppsum,
        ):
            # Warm the PE HAM clock gate while the first input DMAs land.
            wps = warmps.tile([128, 128], F32)
            for i in range(N_WARM):
                nc.tensor.matmul(
                    wps, lhsT=ones_sb, rhs=ones_sb,
                    start=(i == 0), stop=(i == N_WARM - 1),
                )

            # V natural ([krows, dv], krows on partitions): stationary is
            # the x^T block, the weight columns stream.
            for qv in range(NQC):
                xv = xv0 if qv == 0 else x_stream(vT, qv)
                for dci in (range(4) if qv == 0 else []):
                    weight_slice(wk_sb, wk, dci)
                for kbs in range(4):
                    kb = 4 * qv + kbs
                    if kbs == 3 and qv < NQC - 1:
                        for dci in range(4 * (qv + 1), 4 * (qv + 2)):
                            weight_slice(wk_sb, wk, dci)
                    for nn in range(2):
                        ps = ppsum.tile([128, 512], F32, tag="pp")
                        for dci in range(NDC):
                            nc.tensor.matmul(
                                ps,
                                lhsT=xv[:, dci, kbs * 128 : (kbs + 1) * 128],
                                rhs=wv_sb[:, dci, nn * 512 : (nn + 1) * 512],
                                start=(dci == 0),
                                stop=(dci == NDC - 1),
                            )
                        nc.vector.tensor_copy(
                            out=v_all[:, nn * 4 : (nn + 1) * 4, kb, :], in_=ps
                        )

            # K^T per head ([dk, q]): weight slice stationary, x^T streams.
            wq_sb = weight_tile(pool_A)  # reuses the wv zone
            for qc in range(NQC):
                xs = x_stream(kT, qc)
                for dci in range(4 * qc, 4 * qc + 4):
                    weight_slice(wq_sb, wq, dci)
                for h in range(HG):
                    ps = ppsum.tile([128, QC], F32, tag="pp")
                    for dci in range(NDC):
                        nc.tensor.matmul(
                            ps,
                            lhsT=wk_sb[:, dci, h * 128 : (h + 1) * 128],
                            rhs=xs[:, dci, :],
                            start=(dci == 0),
                            stop=(dci == NDC - 1),
                        )
                    nc.vector.tensor_copy(
                        out=k_all[:, h, qc * QC : (qc + 1) * QC], in_=ps
                    )

            # Q projection: chunks 0-1 in full, chunk 2 heads 0-3. The
            # rest (Q2 heads 4-7 with the still-resident xs stream, then
            # all of Q3 via fresh xq streams) is deferred into the
            # attention phase as PE filler - Q2-tail fills have ZERO
            # landing latency, which is what att0 needs.
            xs_q2 = None
            for qc in range(3):
                xs = x_stream(qT, qc)
                if qc == 2:
                    xs_q2 = xs
                for h in range(HG if qc < 2 else 4):
                    ps = ppsum.tile([128, QC], F32, tag="pp")
                    for dci in range(NDC):
                        nc.tensor.matmul(
                            ps,
                            lhsT=wq_sb[:, dci, h * 128 : (h + 1) * 128],
                            rhs=xs[:, dci, :],
                            start=(dci == 0),
                            stop=(dci == NDC - 1),
                        )
                    nc.vector.tensor_copy(
                        out=q_all[:, h, qc * QC : (qc + 1) * QC], in_=ps
                    )

        # Deferred-Q3 half-chunk streams: [128, NDC, 256] tiles drawn
        # from x_pool itself (they fit the existing "xs" slots). Slot
        # rotation makes half 0 land during Q2-proj (its slot freed at
        # Q1-proj end) and half 1 land right after the Q2-tail fills
        # release xs_q2's slot - so the Q3 filler never stalls the PE.
        def xq_stream(half):
            xs = x_pool.tile([128, NDC, QC // 2], BF16, tag="xs", name="xqs")
            (nc.scalar if half == 0 else nc.sync).dma_start(
                out=xs,
                in_=qT[:, 3 * QC + half * 256 : 3 * QC + (half + 1) * 256]
                .rearrange("(o p) f -> p o f", p=128),
            )
            return xs

        xq_tiles = [xq_stream(0), xq_stream(1)]

        # wo lands in the SBUF recycled from the wk zone; its 4MB DMA
        # (scalar ring, behind the xq prefetches) waits for the last wk
        # read (end of K proj) and runs under Q0-Q2/att0; first use is
        # the wo0 filler during att2.
        wo_sb = pool_B.tile([128, NDC, D // 2], BF16, tag="w", name="wo_sb")
        for dci in range(NDC):
            weight_slice(wo_sb, wo, dci, eng=nc.scalar)

        # ---------- Phase 2: attention with filler interleave ----------
        spsum = ent(tc.tile_pool(name="spsum", bufs=2, space="PSUM"))
        opsum = ent(tc.tile_pool(name="opsum", bufs=2, space="PSUM"))
        rpsum = ent(tc.tile_pool(name="rpsum", bufs=2, space="PSUM"))
        fill_ps = ent(tc.tile_pool(name="fillps", bufs=2, space="PSUM"))

        stack_M = ExitStack()  # mfq + ob pools, opened after stack_A closes
        mfq_pool_box = {}

        def wo_load(qc):
            # Prefetch the gathered heads for wo(qc).
            mfq = mfq_pool_box["pool"].tile([128, H, QC], BF16, tag="mfq",
                                            name="mfq")
            if qc < 2:
                nc.gpsimd.dma_start(out=mfq[:, :HG, :], in_=cc_out[qc][0])
                nc.gpsimd.dma_start(out=mfq[:, HG:, :], in_=cc_out[qc][1])
            elif qc == 2:
                for j in range(2):
                    nc.gpsimd.dma_start(
                        out=mfq[:, 4 * j : 4 * j + 4, :], in_=cc2_out[j][0]
                    )
                    nc.gpsimd.dma_start(
                        out=mfq[:, HG + 4 * j : HG + 4 * j + 4, :],
                        in_=cc2_out[j][1],
                    )
            else:
                for j in range(4):
                    nc.gpsimd.dma_start(
                        out=mfq[:, 2 * j : 2 * j + 2, :], in_=cc3_out[j][0]
                    )
                    nc.gpsimd.dma_start(
                        out=mfq[:, HG + 2 * j : HG + 2 * j + 2, :],
                        in_=cc3_out[j][1],
                    )
            return mfq

        def wo_col_gen(qc, col, mfq):
            w_ps = fill_ps.tile([128, QC], F32, tag="fill")
            for hv in range(H):
                nc.tensor.matmul(
                    w_ps,
                    lhsT=wo_sb[:, hv, col * 128 : (col + 1) * 128],
                    rhs=mfq[:, hv, :],
                    start=(hv == 0),
                    stop=(hv == H - 1),
                )
                yield QC
            # drain on DVE and ship outT on the SYNC ring: the scalar
            # engine queue is the exp pipeline - its DMA_DIRECT2D slots
            # (~0.7us each) would add latency to every exp.
            ob = mfq_pool_box["ob"].tile([128, QC], F32, tag="ob", name="ob")
            nc.vector.tensor_copy(out=ob, in_=w_ps)
            # wo3's drains alternate rings: the ACT queue is exp-free by
            # then and splitting halves the end-of-kernel DMA drain.
            eng = nc.scalar if (qc == 3 and col % 2 == 1) else nc.sync
            eng.dma_start(
                out=outT[
                    col * 128 : (col + 1) * 128,
                    qc * QC : (qc + 1) * QC,
                ],
                in_=ob,
            )

        progress = {"q3": False}

        def filler_gen():
            # -- deferred Q2 heads 4-7: their xs stream is still
            # SBUF-resident, so these fills have no landing latency --
            for h in range(4, HG):
                ps = fill_ps.tile([128, QC], F32, tag="fill")
                for dci in range(NDC):
                    nc.tensor.matmul(
                        ps,
                        lhsT=wq_sb[:, dci, h * 128 : (h + 1) * 128],
                        rhs=xs_q2[:, dci, :],
                        start=(dci == 0),
                        stop=(dci == NDC - 1),
                    )
                    yield QC
                nc.vector.tensor_copy(
                    out=q_all[:, h, 2 * QC : 3 * QC], in_=ps
                )
            # -- deferred Q3 projection, in half-chunks of 256 --
            for half in (0, 1):
                xs = xq_tiles[half]
                for h in range(HG):
                    ps = fill_ps.tile([128, QC], F32, tag="fill")
                    for dci in range(NDC):
                        nc.tensor.matmul(
                            ps[:, : QC // 2],
                            lhsT=wq_sb[:, dci, h * 128 : (h + 1) * 128],
                            rhs=xs[:, dci, :],
                            start=(dci == 0),
                            stop=(dci == NDC - 1),
                        )
                        yield QC // 2
                    nc.vector.tensor_copy(
                        out=q_all[
                            :, h,
                            3 * QC + half * 256 : 3 * QC + (half + 1) * 256,
                        ],
                        in_=ps[:, : QC // 2],
                    )
            progress["q3"] = True
            # -- transition: free the x/wq SBUF, open the mfq + ob pools --
            stack_X.close()
            stack_A.close()
            mfq_pool_box["pool"] = stack_M.enter_context(
                tc.tile_pool(name="mfq", bufs=2)
            )
            mfq_pool_box["ob"] = stack_M.enter_context(
                tc.tile_pool(name="ob", bufs=2)
            )
            # -- wo chunks in gather order; the wo0/1/2 columns left
            # after att3 are the runway that covers the last chunk-3
            # pair exchanges --
            for qc in range(NQC):
                mfq = wo_load(qc)
                for col in range(NCOL):
                    yield from wo_col_gen(qc, col, mfq)

        filler = filler_gen()
        fstate = {"done": False}

        def fill(rows):
            while rows > 0 and not fstate["done"]:
                r = next(filler, None)
                if r is None:
                    fstate["done"] = True
                    return
                rows -= r

        def flush_q3():
            while not progress["q3"] and not fstate["done"]:
                fill(QC)

        def att_head(qc, h):
            filling = True
            # att0-att2 showed 82-94% PE feed at 1-block fills, so fill
            # harder there; att3 keeps a lighter rate so the wo tail
            # retains enough runway to cover the last per-head gathers.
            if qc == 0:
                f_full, f_thin = 768, 1024
            elif qc < 3:
                f_full, f_thin = QC, 768
            else:
                f_full, f_thin = 256, QC
            nkb = 4 * (qc + 1)
            ngrp = (nkb + 3) // 4
            # Process the diagonal-crossing k-group FIRST: its thin
            # (128..512-wide) s->exp->o chains then overlap the dense
            # full-width blocks instead of bunching at the head's end.
            # PSUM accumulation is order-independent; the group-first
            # block always has j0=0 either way.
            grp_order = [ngrp - 1] + list(range(ngrp - 1))
            kb_order = [4 * g + j for g in grp_order for j in range(4)
                        if 4 * g + j < nkb]
            o_ps = opsum.tile([128, QC], F32, tag="opsum")
            r_ps = rpsum.tile([128, QC], F32, tag="rpsum")
            racc = None
            ngrp8 = (nkb + 7) // 8
            for kbi, kb in enumerate(kb_order):
                j0 = kb_start(qc, kb)  # first live q col in chunk
                s_ps = spsum.tile([128, QC], F32, tag="spsum")
                nc.tensor.matmul(
                    s_ps[:, j0:],
                    lhsT=k_all[:, h, kb * 128 : (kb + 1) * 128],
                    rhs=q_all[:, h, qc * QC + j0 : (qc + 1) * QC],
                    start=True,
                    stop=True,
                )
                # Filler between the s matmul and its dependent o matmul
                # absorbs the exp round-trip latency; thin diagonal
                # blocks leave a bigger bubble, so fill more.
                if filling:
                    fill(f_full if j0 == 0 else f_thin)
                pt = pt_pool.tile([128, QC], BF16, tag="pt")
                nc.scalar.activation(
                    out=pt[:, j0:],
                    in_=s_ps[:, j0:],
                    func=mybir.ActivationFunctionType.Exp,
                    scale=float(SCALE),
                )
                if j0 < QC and 0 <= kb - 4 * qc:
                    # mask the diagonal-crossing 128 columns
                    nc.vector.tensor_mul(
                        out=pt[:, j0 : j0 + 128],
                        in0=pt[:, j0 : j0 + 128],
                        in1=tri_sb,
                    )
                nc.tensor.matmul(
                    o_ps[:, j0:],
                    lhsT=v_all[:, h, kb, :],
                    rhs=pt[:, j0:],
                    start=(kbi == 0),
                    stop=(kbi == nkb - 1),
                )
                # Denominators: sum ALL exp-blocks of the chunk on DVE
                # (bf16), then ONE full-width ones-matmul per head-chunk.
                # The first block in kb_order always has j0=0, so racc is
                # fully initialized.
                if kbi == 0:
                    racc = racc_pool.tile([128, QC], BF16, tag="racc")
                    nc.vector.tensor_copy(out=racc, in_=pt)
                else:
                    nc.vector.tensor_add(
                        out=racc[:, j0:], in0=racc[:, j0:], in1=pt[:, j0:]
                    )
                if kbi == nkb - 1:
                    nc.tensor.matmul(
                        r_ps, lhsT=ones_sb, rhs=racc, start=True, stop=True,
                    )
            # 1/r on DVE: single-instruction NR-seeded approximation
            # (~51 ULP) - the exact reciprocal held the rpsum bank
            # hostage and stalled the PE.
            rinv = rinv_pool.tile([128, QC], F32, tag="rinv")
            nc.vector.reciprocal_approx_fast(out=rinv, in_=r_ps)
            msb = mstage.tile([128, QC], BF16, tag="mstage")
            nc.vector.tensor_mul(out=msb, in0=o_ps, in1=rinv)
            # Exchange granularity is chosen so the CC stream (which
            # serializes ops at ~25us/MB + ~3us fixed) finishes the
            # last chunk-3 piece right behind att3: chunks 0/1 whole
            # (latency hidden), chunk 2 in halves, chunk 3 in pairs.
            if qc < 2:
                nc.sync.dma_start(out=cc_in[qc][:, h, :], in_=msb)
                if h == HG - 1:
                    nc.gpsimd.collective_compute(
                        "AllGather",
                        mybir.AluOpType.bypass,
                        ins=[cc_in[qc][:]],
                        outs=[cc_out[qc][:]],
                        replica_groups=pair_groups,
                    )
            elif qc == 2:
                nc.sync.dma_start(out=cc2_in[h // 4][:, h % 4, :], in_=msb)
                if h % 4 == 3:
                    nc.gpsimd.collective_compute(
                        "AllGather",
                        mybir.AluOpType.bypass,
                        ins=[cc2_in[h // 4][:]],
                        outs=[cc2_out[h // 4][:]],
                        replica_groups=pair_groups,
                    )
            else:
                nc.sync.dma_start(out=cc3_in[h // 2][:, h % 2, :], in_=msb)
                if h % 2 == 1:
                    nc.gpsimd.collective_compute(
                        "AllGather",
                        mybir.AluOpType.bypass,
                        ins=[cc3_in[h // 2][:]],
                        outs=[cc3_out[h // 2][:]],
                        replica_groups=pair_groups,
                    )

        for qc in range(NQC):
            if qc == 3:
                flush_q3()
            for h in range(HG):
                att_head(qc, h)
        # drain whatever filler remains (dense wo tail)
        while not fstate["done"]:
            fill(QC * 16)
        stack_M.close()

    nc.compile()
    return nc


def build_kernel_legacy(causal: bool):
    """Baseline schedule (kept for the non-causal mask fallback)."""
    nc = bacc.Bacc("TRN2", num_devices=N_CORES)

    qT = nc.declare_dram_parameter("qT", [D, T], BF16, isOutput=False)
    kT = nc.declare_dram_parameter("kT", [D, T], BF16, isOutput=False)
    vT = nc.declare_dram_parameter("vT", [D, T], BF16, isOutput=False)
    wq = nc.declare_dram_parameter("wq", [D, HG * DK], BF16, isOutput=False)
    wk = nc.declare_dram_parameter("wk", [D, HG * DK], BF16, isOutput=False)
    wv = nc.declare_dram_parameter("wv", [D, HG * DV], BF16, isOutput=False)
    wo = nc.declare_dram_parameter("wo", [H * DV, D // 2], BF16, isOutput=False)
    tri = nc.declare_dram_parameter("tri", [128, 128], BF16, isOutput=False)
    if not causal:
        maskT = nc.declare_dram_parameter("maskT", [T, T], BF16, isOutput=False)
    outT = nc.declare_dram_parameter("outT", [D // 2, T], F32, isOutput=True)

    cc_in = {qc: nc.dram_tensor(f"cc_in_{qc}", [128, HG, QC], BF16)
             for qc in range(NQC)}
    cc_out = {qc: nc.dram_tensor(f"cc_out_{qc}", [2, 128, HG, QC], BF16)
              for qc in range(NQC)}
    cc_warm_in = nc.dram_tensor("cc_warm_in", [128, 8], BF16)
    cc_warm_out = nc.dram_tensor("cc_warm_out", [2, 128, 8], BF16)
    pair_groups = [[0, 1], [2, 3], [4, 5], [6, 7]]

    ATT_ORDER = (3, 0, 2, 1)

    def kb_start(qc, kb):
        if not causal:
            return 0
        return min(max((kb - 4 * qc) * 128, 0), QC)

    with tile.TileContext(nc) as tc, ExitStack() as top:
        ent = top.enter_context
        consts = ent(tc.tile_pool(name="consts", bufs=1))
        res_pool = ent(tc.tile_pool(name="res", bufs=1))
        proj_stack = ExitStack()
        w_pool = proj_stack.enter_context(tc.tile_pool(name="w", bufs=2))
        x_pool = proj_stack.enter_context(tc.tile_pool(name="xs", bufs=2))

        ones_sb = consts.tile([128, 128], BF16)
        nc.vector.memset(ones_sb, 1.0)
        tri_sb = consts.tile([128, 128], BF16)
        nc.sync.dma_start(out=tri_sb, in_=tri[:])

        nc.gpsimd.collective_compute(
            "AllGather",
            mybir.AluOpType.bypass,
            ins=[cc_warm_in[:]],
            outs=[cc_warm_out[:]],
            replica_groups=pair_groups,
        )

        q_all = res_pool.tile([128, HG, T], BF16, name="q_all")
        k_all = res_pool.tile([128, HG, T], BF16, name="k_all")
        v_all = res_pool.tile([128, HG, NKB, DV], BF16, name="v_all")

        def weight_tile(pool):
            return pool.tile([128, NDC, HG * 128], BF16, tag="w", name="wtile")

        def weight_slice(w_sb, w_ext, dci):
            nc.sync.dma_start(
                out=w_sb[:, dci, :], in_=w_ext[dci * 128 : (dci + 1) * 128, :]
            )

        wv_sb = weight_tile(w_pool)
        for c in range(4):
            nc.sync.dma_start(
                out=wv_sb[:, 4 * c : 4 * (c + 1), :],
                in_=wv[4 * c * 128 : 4 * (c + 1) * 128, :].rearrange(
                    "(o p) f -> p o f", p=128
                ),
            )
        wk_sb = weight_tile(w_pool)
        wq_sb = None

        def x_stream(src, qc):
            xs = x_pool.tile([128, NDC, QC], BF16, tag="xs")
            nc.scalar.dma_start(
                out=xs,
                in_=src[:, qc * QC : (qc + 1) * QC].rearrange(
                    "(o p) f -> p o f", p=128
                ),
            )
            return xs

        with (
            tc.tile_pool(name="warmps", bufs=1, space="PSUM") as warmps,
            tc.tile_pool(name="ppsum", bufs=4, space="PSUM") as ppsum,
        ):
            wps = warmps.tile([128, 128], F32)
            for i in range(290):
                nc.tensor.matmul(
                    wps, lhsT=ones_sb, rhs=ones_sb,
                    start=(i == 0), stop=(i == 289),
                )

            for qv in range(NQC):
                xv = x_stream(vT, qv)
                for dci in (range(4) if qv == 0 else []):
                    weight_slice(wk_sb, wk, dci)
                for kbs in range(4):
                    kb = 4 * qv + kbs
                    if kbs == 3 and qv < NQC - 1:
                        for dci in range(4 * (qv + 1), 4 * (qv + 2)):
                            weight_slice(wk_sb, wk, dci)
                    for nn in range(2):
                        ps = ppsum.tile([128, 512], F32, tag="pp")
                        for dci in range(NDC):
                            nc.tensor.matmul(
                                ps,
                                lhsT=xv[:, dci, kbs * 128 : (kbs + 1) * 128],
                                rhs=wv_sb[:, dci, nn * 512 : (nn + 1) * 512],
                                start=(dci == 0),
                                stop=(dci == NDC - 1),
                            )
                        nc.vector.tensor_copy(
                            out=v_all[:, nn * 4 : (nn + 1) * 4, kb, :], in_=ps
                        )

            wq_sb = weight_tile(w_pool)
            for qc in range(NQC):
                xs = x_stream(kT, qc)
                for dci in range(4 * qc, 4 * qc + 4):
                    weight_slice(wq_sb, wq, dci)
                for h in range(HG):
                    ps = ppsum.tile([128, QC], F32, tag="pp")
                    for dci in range(NDC):
                        nc.tensor.matmul(
                            ps,
                            lhsT=wk_sb[:, dci, h * 128 : (h + 1) * 128],
                            rhs=xs[:, dci, :],
                            start=(dci == 0),
                            stop=(dci == NDC - 1),
                        )
                    nc.vector.tensor_copy(
                        out=k_all[:, h, qc * QC : (qc + 1) * QC], in_=ps
                    )

            for qc in ATT_ORDER:
                xs = x_stream(qT, qc)
                for h in range(HG):
                    ps = ppsum.tile([128, QC], F32, tag="pp")
                    for dci in range(NDC):
                        nc.tensor.matmul(
                            ps,
                            lhsT=wq_sb[:, dci, h * 128 : (h + 1) * 128],
                            rhs=xs[:, dci, :],
                            start=(dci == 0),
                            stop=(dci == NDC - 1),
                        )
                    nc.vector.tensor_copy(
                        out=q_all[:, h, qc * QC : (qc + 1) * QC], in_=ps
                    )

        proj_stack.close()

        wo_pool = ent(tc.tile_pool(name="wop", bufs=1))
        wo_sb = wo_pool.tile([128, NDC, D // 2], BF16, name="wo_sb")
        for dci in range(NDC):
            weight_slice(wo_sb, wo, dci)
        pt_pool = ent(tc.tile_pool(name="pt", bufs=10 if causal else 6))
        racc_pool = ent(tc.tile_pool(name="racc", bufs=4 if causal else 3))
        mstage = ent(tc.tile_pool(name="mstage", bufs=4 if causal else 3))
        rinv_pool = ent(tc.tile_pool(name="rinv", bufs=3 if causal else 2))
        mfq_pool = ent(tc.tile_pool(name="mfq", bufs=2 if causal else 1))
        ob_pool = ent(tc.tile_pool(name="ob", bufs=4))
        gm_pool = ent(tc.tile_pool(name="gm", bufs=2)) if not causal else None
        spsum = ent(tc.tile_pool(name="spsum", bufs=2, space="PSUM"))
        opsum = ent(tc.tile_pool(name="opsum", bufs=2, space="PSUM"))
        rpsum = ent(tc.tile_pool(name="rpsum", bufs=2, space="PSUM"))
        wpsum = ent(tc.tile_pool(name="wpsum", bufs=2, space="PSUM"))

        def load_gm(qc):
            if causal:
                return None
            gm = gm_pool.tile([128, NKB, QC], BF16, tag="gm")
            nc.scalar.dma_start(
                out=gm,
                in_=maskT[:, qc * QC : (qc + 1) * QC].rearrange(
                    "(o p) f -> p o f", p=128
                ),
            )
            return gm

        def att_head(qc, h, gm):
            nkb = 4 * (qc + 1) if causal else NKB
            ngrp = (nkb + 3) // 4
            grp_order = ([ngrp - 1] + list(range(ngrp - 1))) if causal else \
                list(range(ngrp))
            kb_order = [4 * g + j for g in grp_order for j in range(4)
                        if 4 * g + j < nkb]
            o_ps = opsum.tile([128, QC], F32, tag="opsum")
            r_ps = rpsum.tile([128, QC], F32, tag="rpsum")
            racc = None
            for kbi, kb in enumerate(kb_order):
                j0 = kb_start(qc, kb)
                s_ps = spsum.tile([128, QC], F32, tag="spsum")
                nc.tensor.matmul(
                    s_ps[:, j0:],
                    lhsT=k_all[:, h, kb * 128 : (kb + 1) * 128],
                    rhs=q_all[:, h, qc * QC + j0 : (qc + 1) * QC],
                    start=True,
                    stop=True,
                )
                pt = pt_pool.tile([128, QC], BF16, tag="pt")
                nc.scalar.activation(
                    out=pt[:, j0:],
                    in_=s_ps[:, j0:],
                    func=mybir.ActivationFunctionType.Exp,
                    scale=float(SCALE),
                )
                if causal:
                    if j0 < QC and kb - 4 * qc >= 0:
                        nc.vector.tensor_mul(
                            out=pt[:, j0 : j0 + 128],
                            in0=pt[:, j0 : j0 + 128],
                            in1=tri_sb,
                        )
                else:
                    nc.vector.tensor_mul(out=pt, in0=pt, in1=gm[:, kb, :])
                nc.tensor.matmul(
                    o_ps[:, j0:],
                    lhsT=v_all[:, h, kb, :],
                    rhs=pt[:, j0:],
                    start=(kbi == 0),
                    stop=(kbi == nkb - 1),
                )
                if kbi % 4 == 0:
                    racc = racc_pool.tile([128, QC], BF16, tag="racc")
                    nc.vector.tensor_copy(out=racc, in_=pt)
                else:
                    nc.vector.tensor_add(
                        out=racc[:, j0:], in0=racc[:, j0:], in1=pt[:, j0:]
                    )
                if kbi % 4 == 3 or kbi == nkb - 1:
                    nc.tensor.matmul(
                        r_ps,
                        lhsT=ones_sb,
                        rhs=racc,
                        start=(kbi // 4 == 0),
                        stop=(kbi // 4 == ngrp - 1),
                    )
            rinv = rinv_pool.tile([128, QC], F32, tag="rinv")
            nc.vector.reciprocal_approx_fast(out=rinv, in_=r_ps)
            msb = mstage.tile([128, QC], BF16, tag="mstage")
            nc.vector.tensor_mul(out=msb, in0=o_ps, in1=rinv)
            nc.sync.dma_start(out=cc_in[qc][:, h, :], in_=msb)
            if h == HG - 1:
                nc.gpsimd.collective_compute(
                    "AllGather",
                    mybir.AluOpType.bypass,
                    ins=[cc_in[qc][:]],
                    outs=[cc_out[qc][:]],
                    replica_groups=pair_groups,
                )

        def wo_load(qc):
            mfq = mfq_pool.tile([128, H, QC], BF16, tag="mfq")
            nc.sync.dma_start(out=mfq[:, :HG, :], in_=cc_out[qc][0])
            nc.sync.dma_start(out=mfq[:, HG:, :], in_=cc_out[qc][1])
            return mfq

        def wo_col(qc, col, mfq):
            w_ps = wpsum.tile([128, QC], F32, tag="wpsum")
            for hv in range(H):
                nc.tensor.matmul(
                    w_ps,
                    lhsT=wo_sb[:, hv, col * 128 : (col + 1) * 128],
                    rhs=mfq[:, hv, :],
                    start=(hv == 0),
                    stop=(hv == H - 1),
                )
            ob = ob_pool.tile([128, QC], F32, tag="ob")
            nc.scalar.activation(
                out=ob, in_=w_ps, func=mybir.ActivationFunctionType.Copy
            )
            nc.scalar.dma_start(
                out=outT[
                    col * 128 : (col + 1) * 128,
                    qc * QC : (qc + 1) * QC,
                ],
                in_=ob,
            )

        gm3 = load_gm(3)
        for h in range(HG):
            att_head(3, h, gm3)
        gm0 = load_gm(0)
        for h in range(HG):
            att_head(0, h, gm0)
        mfq3 = wo_load(3)
        gm2 = load_gm(2)
        for h in range(HG):
            att_head(2, h, gm2)
        mfq0 = wo_load(0)
        gm1 = load_gm(1)
        for h in range(HG):
            att_head(1, h, gm1)
            wo_col(3, h, mfq3)
        mfq2 = wo_load(2)
        for col in range(NCOL):
            wo_col(0, col, mfq0)
        mfq1 = wo_load(1)
        for col in range(NCOL):
            wo_col(2, col, mfq2)
        for col in range(NCOL):
            wo_col(1, col, mfq1)

    nc.compile()
    return nc


def kernel(q, k, v, mask, Wq, Wk, Wv, Wo):
    q = np.asarray(q)
    k = np.asarray(k)
    v = np.asarray(v)
    mask = np.asarray(mask)
    causal = bool(np.array_equal(mask, np.tril(np.ones((T, T), dtype=bool))))

    if causal not in _KERNEL_CACHE:
        _KERNEL_CACHE[causal] = (
            build_kernel_causal() if causal else build_kernel_legacy(False)
        )
    nc = _KERNEL_CACHE[causal]

    bf = ml_dtypes.bfloat16
    Wq_b = np.asarray(Wq).astype(bf)
    Wk_b = np.asarray(Wk).astype(bf)
    Wv_b = np.asarray(Wv).astype(bf)
    Wo_b = np.asarray(Wo).astype(bf)
    i = np.arange(128)
    tri_np = (i[None, :] >= i[:, None]).astype(bf)  # tri[k, j] = j >= k
    maskT_np = None if causal else np.ascontiguousarray(mask.T).astype(bf)

    in_maps = []
    for c in range(N_CORES):
        b, g = c // 2, c % 2
        m = {
            "qT": np.ascontiguousarray(q[b].T).astype(bf),
            "kT": np.ascontiguousarray(k[b].T).astype(bf),
            "vT": np.ascontiguousarray(v[b].T).astype(bf),
            "wq": np.ascontiguousarray(Wq_b[:, g * 1024 : (g + 1) * 1024]),
            "wk": np.ascontiguousarray(Wk_b[:, g * 1024 : (g + 1) * 1024]),
            "wv": np.ascontiguousarray(Wv_b[:, g * 1024 : (g + 1) * 1024]),
            "wo": np.ascontiguousarray(Wo_b[:, g * 1024 : (g + 1) * 1024]),
            "tri": tri_np,
        }
        if not causal:
            m["maskT"] = maskT_np
        in_maps.append(m)

    trace = bool(os.environ.get("BASS_KERNEL_TRACE")) and (
        "antenv.axon_hooks" in sys.modules
    )
    res = run_bass_kernel_spmd(nc, in_maps, list(range(N_CORES)), trace=trace)
    if trace and res.exec_time_ns is not None:
        print(f"HW exec time: {res.exec_time_ns} ns")
        kernel.last_exec_time_ns = res.exec_time_ns
        kernel.last_results = res

    out = np.empty((B, T, D), dtype=np.float32)
    for b in range(B):
        top = res.results[2 * b]["outT"]        # cols 0..1023, [1024, 2048]
        bot = res.results[2 * b + 1]["outT"]    # cols 1024..2047
        out[b] = np.concatenate([top, bot], axis=0).T
    return out


# revision 32
# speedup vs baseline: 1.0042x; 1.0042x over previous
"""Distributed multi-head causal attention for 8 TRN2 NeuronCores.

Problem: B=4, T=2048, D=2048, H=16 heads of dk=dv=128.
  out = softmax(mask((q@Wq)(k@Wk)^T / sqrt(dk))) @ (v@Wv) @ Wo

Sharding (2D; all per-core asymmetry lives in host-supplied data so the
SPMD graph is identical on every core):
  core c -> batch b = c//2, head-group g = c%2 (heads 8g..8g+7).
  - QKV projections + attention for (batch b, its 8 heads): fully local.
  - Pair AllGather (replica groups [2b, 2b+1]) exchanges the per-head
    attention outputs (merged^T, bf16) per q-chunk.
  - Output projection: each core computes out^T for its batch for HALF
    the output columns (even core: cols 0..1023, odd: 1024..2047).
  Host reassembles: out[b] = concat(outT_2b, outT_2b+1, axis=0).T

Performance structure (v4):
  - All intermediates (Q^T/K^T per head, V natural) stay RESIDENT IN
    SBUF - no DRAM round trip, no attention-phase input DMAs.
  - Attention chunks run IN ORDER (0,1,2,3). The per-block s->exp->o
    latency bubbles of the in-order PE queue are absorbed by a FILLER
    stream of independent matmuls interleaved between attention
    matmuls: the deferred Q2-tail heads (xs stream still SBUF-resident,
    zero landing latency - fills att0), then the deferred Q3 projection
    (half-chunk xq streams drawn from x_pool slots that free early),
    then the wo output projections of already-gathered chunks, ending
    in a dense wo tail whose runway covers the last exchanges.
  - Pair-exchange granularity matches the CC stream's serial cost
    (~25us/MB + ~3-10us fixed per op): chunks 0/1 gather whole (latency
    hidden), chunk 2 in halves, chunk 3 in head-pairs so the last
    256KB piece lands right behind att3 for the wo3 tail.
  - wv is spread over the sync/gpsimd DGE rings (and xv0 leads the
    scalar ring) so the first V-proj accumulation (which needs all 16
    k-slices of wv) never stalls on ring bootstrap; N_WARM dummy
    matmuls cover that window and keep the PE HAM clock gate at K=8/8.
  - mfq loads ride the otherwise-idle gpsimd ring and outT drains ride
    sync, keeping the scalar engine queue free for the exp pipeline.

Compute is bf16 on TensorE with f32 PSUM accumulation. Softmax skips the
max-subtraction (scores are ~N(0,1); exp is safe in f32) and obtains the
denominators with a ones-matmul per 8 exp-blocks (DVE-accumulated bf16
partial sums); causal masking multiplies exp(scores) by a 0/1 triangular
tile on the single diagonal-crossing 128x128 sub-block, and the moving
free dim of diagonal-region matmuls is trimmed to the unmasked columns.
"""
import os
import sys
from contextlib import ExitStack

import numpy as np
import ml_dtypes

import concourse.bass as bass
import concourse.mybir as mybir
import concourse.tile as tile
from concourse import bacc
from concourse.bass_utils import run_bass_kernel_spmd

BF16 = mybir.dt.bfloat16
F32 = mybir.dt.float32

B, T, D = 4, 2048, 2048
H, DK, DV = 16, 128, 128
HG = 8                      # heads per core
N_CORES = 8
QC = 512                    # q-chunk (matmul moving free dim)
NQC = T // QC               # 4
NKB = T // 128              # 16 k-blocks
NDC = D // 128              # 16 contraction chunks
NCOL = D // 2 // 128        # 8 output-projection column blocks per core
SCALE = 1.0 / np.sqrt(DK)
N_WARM = 300                # dummy matmuls to warm the PE clock gate

_KERNEL_CACHE = {}


def build_kernel_causal():
    nc = bacc.Bacc("TRN2", num_devices=N_CORES)

    qT = nc.declare_dram_parameter("qT", [D, T], BF16, isOutput=False)
    kT = nc.declare_dram_parameter("kT", [D, T], BF16, isOutput=False)
    vT = nc.declare_dram_parameter("vT", [D, T], BF16, isOutput=False)
    wq = nc.declare_dram_parameter("wq", [D, HG * DK], BF16, isOutput=False)
    wk = nc.declare_dram_parameter("wk", [D, HG * DK], BF16, isOutput=False)
    wv = nc.declare_dram_parameter("wv", [D, HG * DV], BF16, isOutput=False)
    wo = nc.declare_dram_parameter("wo", [H * DV, D // 2], BF16, isOutput=False)
    tri = nc.declare_dram_parameter("tri", [128, 128], BF16, isOutput=False)
    outT = nc.declare_dram_parameter("outT", [D // 2, T], F32, isOutput=True)

    # Collective staging (collectives require DRAM in/out). Chunks 0-2:
    # ONE pair AllGather per q-chunk (in [128, HG, QC]) - their ~30us
    # fire-to-land latency is hidden by the schedule. Chunk 3 is split
    # into per-HEAD-PAIR gathers (4 x 256KB, fired after h1/h3/h5/h7):
    # the last one lands ~10us after the last head instead of ~33us,
    # which is what the wo3 tail actually waits on. (Fully per-head
    # gathers were tried and are CC-stream throughput-bound: ~6.7us
    # fixed stream occupancy per op x 32 ops backlogs the stream.)
    cc_in = {qc: nc.dram_tensor(f"cc_in_{qc}", [128, HG, QC], BF16)
             for qc in range(2)}
    cc_out = {qc: nc.dram_tensor(f"cc_out_{qc}", [2, 128, HG, QC], BF16)
              for qc in range(2)}
    cc2_in = {j: nc.dram_tensor(f"cc2_in_{j}", [128, 4, QC], BF16)
              for j in range(2)}
    cc2_out = {j: nc.dram_tensor(f"cc2_out_{j}", [2, 128, 4, QC], BF16)
               for j in range(2)}
    cc3_in = {j: nc.dram_tensor(f"cc3_in_{j}", [128, 2, QC], BF16)
              for j in range(4)}
    cc3_out = {j: nc.dram_tensor(f"cc3_out_{j}", [2, 128, 2, QC], BF16)
               for j in range(4)}
    cc_warm_in = nc.dram_tensor("cc_warm_in", [128, 8], BF16)
    cc_warm_out = nc.dram_tensor("cc_warm_out", [2, 128, 8], BF16)
    pair_groups = [[0, 1], [2, 3], [4, 5], [6, 7]]

    def kb_start(qc, kb):
        """First unmasked q column (within the chunk) for this k-block."""
        return min(max((kb - 4 * qc) * 128, 0), QC)

    with tile.TileContext(nc) as tc, ExitStack() as top:
        ent = top.enter_context
        # Pool releases must be LIFO (stack allocator), so creation
        # order is release-reverse: live-to-end pools first (consts,
        # res, B, the attention pools), then A (wv->wq, dies at
        # Q3-drain), then the phase-1 x streams (die at phase-1 end).
        # xq / mfq+ob are pushed later at the then-top of the stack.
        consts = ent(tc.tile_pool(name="consts", bufs=1))
        # SBUF-resident per-head projections (live for the whole kernel).
        res_pool = ent(tc.tile_pool(name="res", bufs=1))
        # Weight zones: A carries wv then (reused) wq; B carries wk then
        # (reused) wo. A closes mid-phase-2 (after the deferred Q3 proj
        # is done); B lives to the end.
        pool_B = ent(tc.tile_pool(name="wB", bufs=1))
        pt_pool = ent(tc.tile_pool(name="pt", bufs=4))
        racc_pool = ent(tc.tile_pool(name="racc", bufs=2))
        mstage = ent(tc.tile_pool(name="mstage", bufs=2))
        rinv_pool = ent(tc.tile_pool(name="rinv", bufs=2))
        stack_A = ExitStack()
        pool_A = stack_A.enter_context(tc.tile_pool(name="wA", bufs=1))
        # Phase-1-only: double-buffered activation streams.
        stack_X = ExitStack()
        x_pool = stack_X.enter_context(tc.tile_pool(name="xs", bufs=2))

        ones_sb = consts.tile([128, 128], BF16)
        nc.vector.memset(ones_sb, 1.0)
        tri_sb = consts.tile([128, 128], BF16)
        nc.sync.dma_start(out=tri_sb, in_=tri[:])

        # Warm the CC stream: the first collective of a NEFF pays the
        # stream barrier + cold-start (~10-30us extra). Fire it now so
        # that happens under the projection phase.
        nc.gpsimd.collective_compute(
            "AllGather",
            mybir.AluOpType.bypass,
            ins=[cc_warm_in[:]],
            outs=[cc_warm_out[:]],
            replica_groups=pair_groups,
        )

        q_all = res_pool.tile([128, HG, T], BF16, name="q_all")
        k_all = res_pool.tile([128, HG, T], BF16, name="k_all")
        v_all = res_pool.tile([128, HG, NKB, DV], BF16, name="v_all")

        def weight_tile(pool):
            return pool.tile([128, NDC, HG * 128], BF16, tag="w", name="wtile")

        def weight_slice(w_sb, w_ext, dci, eng=None):
            (eng or nc.sync).dma_start(
                out=w_sb[:, dci, :], in_=w_ext[dci * 128 : (dci + 1) * 128, :]
            )

        # wv is the startup-critical load: the first V-proj accumulation
        # needs all 16 k-slices, and every DMA ring takes ~10-15us to
        # bootstrap at kernel start. Spread the four 1MB chunks across
        # the THREE DGE rings (sync x2 / gpsimd / scalar-behind-xv0) so
        # they all land by ~20us, inside the warmup window.
        wv_sb = weight_tile(pool_A)
        for c, eng in ((0, nc.sync), (1, nc.sync), (2, nc.gpsimd),
                       (3, nc.sync)):
            eng.dma_start(
                out=wv_sb[:, 4 * c : 4 * (c + 1), :],
                in_=wv[4 * c * 128 : 4 * (c + 1) * 128, :].rearrange(
                    "(o p) f -> p o f", p=128
                ),
            )

        def x_stream(src, qc):
            """[128, NDC, QC] slice of an x^T input, contraction on
            partitions, via the ACT HWDGE ring."""
            xs = x_pool.tile([128, NDC, QC], BF16, tag="xs")
            nc.scalar.dma_start(
                out=xs,
                in_=src[:, qc * QC : (qc + 1) * QC].rearrange(
                    "(o p) f -> p o f", p=128
                ),
            )
            return xs

        xv0 = x_stream(vT, 0)
        wk_sb = weight_tile(pool_B)
        wq_sb = None  # allocated after V proj (reuses wv zone)

        # ------------- Phase 1: V, K, Q0-Q2 projections -------------
        with (
            tc.tile_pool(name="warmps", bufs=1, space="PSUM") as warmps,
            tc.tile_pool(name="ppsum", bufs=4, space="PSUM") as ppsum,
        ):
            # Warm the PE HAM clock gate while the first input DMAs land.
            wps = warmps.tile([128, 128], F32)
            for i in range(N_WARM):
                nc.tensor.matmul(
                    wps, lhsT=ones_sb, rhs=ones_sb,
                    start=(i == 0), stop=(i == N_WARM - 1),
                )

            # V natural ([krows, dv], krows on partitions): stationary is
            # the x^T block, the weight columns stream.
            for qv in range(NQC):
                xv = xv0 if qv == 0 else x_stream(vT, qv)
                for dci in (range(4) if qv == 0 else []):
                    weight_slice(wk_sb, wk, dci)
                for kbs in range(4):
                    kb = 4 * qv + kbs
                    if kbs == 3 and qv < NQC - 1:
                        for dci in range(4 * (qv + 1), 4 * (qv + 2)):
                            weight_slice(wk_sb, wk, dci)
                    for nn in range(2):
                        ps = ppsum.tile([128, 512], F32, tag="pp")
                        for dci in range(NDC):
                            nc.tensor.matmul(
                                ps,
                                lhsT=xv[:, dci, kbs * 128 : (kbs + 1) * 128],
                                rhs=wv_sb[:, dci, nn * 512 : (nn + 1) * 512],
                                start=(dci == 0),
                                stop=(dci == NDC - 1),
                            )
                        nc.vector.tensor_copy(
                            out=v_all[:, nn * 4 : (nn + 1) * 4, kb, :], in_=ps
                        )

            # K^T per head ([dk, q]): weight slice stationary, x^T streams.
            wq_sb = weight_tile(pool_A)  # reuses the wv zone
            for qc in range(NQC):
                xs = x_stream(kT, qc)
                for dci in range(4 * qc, 4 * qc + 4):
                    weight_slice(wq_sb, wq, dci)
                for h in range(HG):
                    ps = ppsum.tile([128, QC], F32, tag="pp")
                    for dci in range(NDC):
                        nc.tensor.matmul(
                            ps,
                            lhsT=wk_sb[:, dci, h * 128 : (h + 1) * 128],
                            rhs=xs[:, dci, :],
                            start=(dci == 0),
                            stop=(dci == NDC - 1),
                        )
                    nc.vector.tensor_copy(
                        out=k_all[:, h, qc * QC : (qc + 1) * QC], in_=ps
                    )

            # Q projection: chunks 0-1 in full, chunk 2 heads 0-3. The
            # rest (Q2 heads 4-7 with the still-resident xs stream, then
            # all of Q3 via fresh xq streams) is deferred into the
            # attention phase as PE filler - Q2-tail fills have ZERO
            # landing latency, which is what att0 needs.
            xs_q2 = None
            for qc in range(3):
                xs = x_stream(qT, qc)
                if qc == 2:
                    xs_q2 = xs
                for h in range(HG if qc < 2 else 4):
                    ps = ppsum.tile([128, QC], F32, tag="pp")
                    for dci in range(NDC):
                        nc.tensor.matmul(
                            ps,
                            lhsT=wq_sb[:, dci, h * 128 : (h + 1) * 128],
                            rhs=xs[:, dci, :],
                            start=(dci == 0),
                            stop=(dci == NDC - 1),
                        )
                    nc.vector.tensor_copy(
                        out=q_all[:, h, qc * QC : (qc + 1) * QC], in_=ps
                    )

        # Deferred-Q3 half-chunk streams: [128, NDC, 256] tiles drawn
        # from x_pool itself (they fit the existing "xs" slots). Slot
        # rotation makes half 0 land during Q2-proj (its slot freed at
        # Q1-proj end) and half 1 land right after the Q2-tail fills
        # release xs_q2's slot - so the Q3 filler never stalls the PE.
        def xq_stream(half):
            xs = x_pool.tile([128, NDC, QC // 2], BF16, tag="xs", name="xqs")
            (nc.scalar if half == 0 else nc.sync).dma_start(
                out=xs,
                in_=qT[:, 3 * QC + half * 256 : 3 * QC + (half + 1) * 256]
                .rearrange("(o p) f -> p o f", p=128),
            )
            return xs

        xq_tiles = [xq_stream(0), xq_stream(1)]

        # wo lands in the SBUF recycled from the wk zone; its 4MB DMA
        # (scalar ring, behind the xq prefetches) waits for the last wk
        # read (end of K proj) and runs under Q0-Q2/att0; first use is
        # the wo0 filler during att2.
        wo_sb = pool_B.tile([128, NDC, D // 2], BF16, tag="w", name="wo_sb")
        for dci in range(NDC):
            weight_slice(wo_sb, wo, dci, eng=nc.scalar)

        # ---------- Phase 2: attention with filler interleave ----------
        spsum = ent(tc.tile_pool(name="spsum", bufs=2, space="PSUM"))
        opsum = ent(tc.tile_pool(name="opsum", bufs=2, space="PSUM"))
        rpsum = ent(tc.tile_pool(name="rpsum", bufs=2, space="PSUM"))
        fill_ps = ent(tc.tile_pool(name="fillps", bufs=2, space="PSUM"))

        stack_M = ExitStack()  # mfq + ob pools, opened after stack_A closes
        mfq_pool_box = {}

        def wo_load(qc):
            # Prefetch the gathered heads for wo(qc).
            mfq = mfq_pool_box["pool"].tile([128, H, QC], BF16, tag="mfq",
                                            name="mfq")
            if qc < 2:
                nc.gpsimd.dma_start(out=mfq[:, :HG, :], in_=cc_out[qc][0])
                nc.gpsimd.dma_start(out=mfq[:, HG:, :], in_=cc_out[qc][1])
            elif qc == 2:
                for j in range(2):
                    nc.gpsimd.dma_start(
                        out=mfq[:, 4 * j : 4 * j + 4, :], in_=cc2_out[j][0]
                    )
                    nc.gpsimd.dma_start(
                        out=mfq[:, HG + 4 * j : HG + 4 * j + 4, :],
                        in_=cc2_out[j][1],
                    )
            else:
                for j in range(4):
                    nc.gpsimd.dma_start(
                        out=mfq[:, 2 * j : 2 * j + 2, :], in_=cc3_out[j][0]
                    )
                    nc.gpsimd.dma_start(
                        out=mfq[:, HG + 2 * j : HG + 2 * j + 2, :],
                        in_=cc3_out[j][1],
                    )
            return mfq

        def wo_col_gen(qc, col, mfq):
            w_ps = fill_ps.tile([128, QC], F32, tag="fill")
            for hv in range(H):
                nc.tensor.matmul(
                    w_ps,
                    lhsT=wo_sb[:, hv, col * 128 : (col + 1) * 128],
                    rhs=mfq[:, hv, :],
                    start=(hv == 0),
                    stop=(hv == H - 1),
                )
                yield QC
            # drain on DVE and ship outT on the SYNC ring: the scalar
            # engine queue is the exp pipeline - its DMA_DIRECT2D slots
            # (~0.7us each) would add latency to every exp.
            ob = mfq_pool_box["ob"].tile([128, QC], F32, tag="ob", name="ob")
            nc.vector.tensor_copy(out=ob, in_=w_ps)
            # wo3's drains alternate rings: the ACT queue is exp-free by
            # then and splitting halves the end-of-kernel DMA drain.
            eng = nc.scalar if (qc == 3 and col % 2 == 1) else nc.sync
            eng.dma_start(
                out=outT[
                    col * 128 : (col + 1) * 128,
                    qc * QC : (qc + 1) * QC,
                ],
                in_=ob,
            )

        progress = {"q3": False}

        def filler_gen():
            # -- deferred Q2 heads 4-7: their xs stream is still
            # SBUF-resident, so these fills have no landing latency --
            for h in range(4, HG):
                ps = fill_ps.tile([128, QC], F32, tag="fill")
                for dci in range(NDC):
                    nc.tensor.matmul(
                        ps,
                        lhsT=wq_sb[:, dci, h * 128 : (h + 1) * 128],
                        rhs=xs_q2[:, dci, :],
                        start=(dci == 0),
                        stop=(dci == NDC - 1),
                    )
                    yield QC
                nc.vector.tensor_copy(
                    out=q_all[:, h, 2 * QC : 3 * QC], in_=ps
                )
            # -- deferred Q3 projection, in half-chunks of 256 --
            for half in (0, 1):
                xs = xq_tiles[half]
                for h in range(HG):
                    ps = fill_ps.tile([128, QC], F32, tag="fill")
                    for dci in range(NDC):
                        nc.tensor.matmul(
                            ps[:, : QC // 2],
                            lhsT=wq_sb[:, dci, h * 128 : (h + 1) * 128],
                            rhs=xs[:, dci, :],
                            start=(dci == 0),
                            stop=(dci == NDC - 1),
                        )
                        yield QC // 2
                    nc.vector.tensor_copy(
                        out=q_all[
                            :, h,
                            3 * QC + half * 256 : 3 * QC + (half + 1) * 256,
                        ],
                        in_=ps[:, : QC // 2],
                    )
            progress["q3"] = True
            # -- transition: free the x/wq SBUF, open the mfq + ob pools --
            stack_X.close()
            stack_A.close()
            mfq_pool_box["pool"] = stack_M.enter_context(
                tc.tile_pool(name="mfq", bufs=2)
            )
            mfq_pool_box["ob"] = stack_M.enter_context(
                tc.tile_pool(name="ob", bufs=2)
            )
            # -- wo chunks in gather order; the wo0/1/2 columns left
            # after att3 are the runway that covers the last chunk-3
            # pair exchanges --
            for qc in range(NQC):
                mfq = wo_load(qc)
                for col in range(NCOL):
                    yield from wo_col_gen(qc, col, mfq)

        filler = filler_gen()
        fstate = {"done": False}

        def fill(rows):
            while rows > 0 and not fstate["done"]:
                r = next(filler, None)
                if r is None:
                    fstate["done"] = True
                    return
                rows -= r

        def flush_q3():
            while not progress["q3"] and not fstate["done"]:
                fill(QC)

        def att_head(qc, h):
            filling = True
            # att0-att2 showed 82-94% PE feed at 1-block fills, so fill
            # harder there; att3 keeps a lighter rate so the wo tail
            # retains enough runway to cover the last per-head gathers.
            if qc == 0:
                f_full, f_thin = 768, 1024
            elif qc < 3:
                f_full, f_thin = QC, 768
            else:
                f_full, f_thin = 256, QC
            nkb = 4 * (qc + 1)
            ngrp = (nkb + 3) // 4
            # Process the diagonal-crossing k-group FIRST: its thin
            # (128..512-wide) s->exp->o chains then overlap the dense
            # full-width blocks instead of bunching at the head's end.
            # PSUM accumulation is order-independent; the group-first
            # block always has j0=0 either way.
            grp_order = [ngrp - 1] + list(range(ngrp - 1))
            kb_order = [4 * g + j for g in grp_order for j in range(4)
                        if 4 * g + j < nkb]
            o_ps = opsum.tile([128, QC], F32, tag="opsum")
            r_ps = rpsum.tile([128, QC], F32, tag="rpsum")
            racc = None
            for kbi, kb in enumerate(kb_order):
                j0 = kb_start(qc, kb)  # first live q col in chunk
                s_ps = spsum.tile([128, QC], F32, tag="spsum")
                nc.tensor.matmul(
                    s_ps[:, j0:],
                    lhsT=k_all[:, h, kb * 128 : (kb + 1) * 128],
                    rhs=q_all[:, h, qc * QC + j0 : (qc + 1) * QC],
                    start=True,
                    stop=True,
                )
                # Filler between the s matmul and its dependent o matmul
                # absorbs the exp round-trip latency; thin diagonal
                # blocks leave a bigger bubble, so fill more.
                if filling:
                    fill(f_full if j0 == 0 else f_thin)
                pt = pt_pool.tile([128, QC], BF16, tag="pt")
                nc.scalar.activation(
                    out=pt[:, j0:],
                    in_=s_ps[:, j0:],
                    func=mybir.ActivationFunctionType.Exp,
                    scale=float(SCALE),
                )
                if j0 < QC and 0 <= kb - 4 * qc:
                    # mask the diagonal-crossing 128 columns
                    nc.vector.tensor_mul(
                        out=pt[:, j0 : j0 + 128],
                        in0=pt[:, j0 : j0 + 128],
                        in1=tri_sb,
                    )
                nc.tensor.matmul(
                    o_ps[:, j0:],
                    lhsT=v_all[:, h, kb, :],
                    rhs=pt[:, j0:],
                    start=(kbi == 0),
                    stop=(kbi == nkb - 1),
                )
                # Denominators: sum ALL exp-blocks of the chunk on DVE
                # (bf16), then ONE full-width ones-matmul per head-chunk.
                # The first block in kb_order always has j0=0, so racc is
                # fully initialized.
                if kbi == 0:
                    racc = racc_pool.tile([128, QC], BF16, tag="racc")
                    nc.vector.tensor_copy(out=racc, in_=pt)
                else:
                    nc.vector.tensor_add(
                        out=racc[:, j0:], in0=racc[:, j0:], in1=pt[:, j0:]
                    )
                if kbi == nkb - 1:
                    nc.tensor.matmul(
                        r_ps, lhsT=ones_sb, rhs=racc, start=True, stop=True,
                    )
            # 1/r on DVE: single-instruction NR-seeded approximation
            # (~51 ULP) - the exact reciprocal held the rpsum bank
            # hostage and stalled the PE.
            rinv = rinv_pool.tile([128, QC], F32, tag="rinv")
            nc.vector.reciprocal_approx_fast(out=rinv, in_=r_ps)
            msb = mstage.tile([128, QC], BF16, tag="mstage")
            nc.vector.tensor_mul(out=msb, in0=o_ps, in1=rinv)
            # Exchange granularity is chosen so the CC stream (which
            # serializes ops at ~25us/MB + ~3us fixed) finishes the
            # last chunk-3 piece right behind att3: chunks 0/1 whole
            # (latency hidden), chunk 2 in halves, chunk 3 in pairs.
            if qc < 2:
                nc.sync.dma_start(out=cc_in[qc][:, h, :], in_=msb)
                if h == HG - 1:
                    nc.gpsimd.collective_compute(
                        "AllGather",
                        mybir.AluOpType.bypass,
                        ins=[cc_in[qc][:]],
                        outs=[cc_out[qc][:]],
                        replica_groups=pair_groups,
                    )
            elif qc == 2:
                nc.sync.dma_start(out=cc2_in[h // 4][:, h % 4, :], in_=msb)
                if h % 4 == 3:
                    nc.gpsimd.collective_compute(
                        "AllGather",
                        mybir.AluOpType.bypass,
                        ins=[cc2_in[h // 4][:]],
                        outs=[cc2_out[h // 4][:]],
                        replica_groups=pair_groups,
                    )
            else:
                nc.sync.dma_start(out=cc3_in[h // 2][:, h % 2, :], in_=msb)
                if h % 2 == 1:
                    nc.gpsimd.collective_compute(
                        "AllGather",
                        mybir.AluOpType.bypass,
                        ins=[cc3_in[h // 2][:]],
                        outs=[cc3_out[h // 2][:]],
                        replica_groups=pair_groups,
                    )

        for qc in range(NQC):
            if qc == 3:
                flush_q3()
            for h in range(HG):
                att_head(qc, h)
        # drain whatever filler remains (dense wo tail)
        while not fstate["done"]:
            fill(QC * 16)
        stack_M.close()

    nc.compile()
    return nc


def build_kernel_legacy(causal: bool):
    """Baseline schedule (kept for the non-causal mask fallback)."""
    nc = bacc.Bacc("TRN2", num_devices=N_CORES)

    qT = nc.declare_dram_parameter("qT", [D, T], BF16, isOutput=False)
    kT = nc.declare_dram_parameter("kT", [D, T], BF16, isOutput=False)
    vT = nc.declare_dram_parameter("vT", [D, T], BF16, isOutput=False)
    wq = nc.declare_dram_parameter("wq", [D, HG * DK], BF16, isOutput=False)
    wk = nc.declare_dram_parameter("wk", [D, HG * DK], BF16, isOutput=False)
    wv = nc.declare_dram_parameter("wv", [D, HG * DV], BF16, isOutput=False)
    wo = nc.declare_dram_parameter("wo", [H * DV, D // 2], BF16, isOutput=False)
    tri = nc.declare_dram_parameter("tri", [128, 128], BF16, isOutput=False)
    if not causal:
        maskT = nc.declare_dram_parameter("maskT", [T, T], BF16, isOutput=False)
    outT = nc.declare_dram_parameter("outT", [D // 2, T], F32, isOutput=True)

    cc_in = {qc: nc.dram_tensor(f"cc_in_{qc}", [128, HG, QC], BF16)
             for qc in range(NQC)}
    cc_out = {qc: nc.dram_tensor(f"cc_out_{qc}", [2, 128, HG, QC], BF16)
              for qc in range(NQC)}
    cc_warm_in = nc.dram_tensor("cc_warm_in", [128, 8], BF16)
    cc_warm_out = nc.dram_tensor("cc_warm_out", [2, 128, 8], BF16)
    pair_groups = [[0, 1], [2, 3], [4, 5], [6, 7]]

    ATT_ORDER = (3, 0, 2, 1)

    def kb_start(qc, kb):
        if not causal:
            return 0
        return min(max((kb - 4 * qc) * 128, 0), QC)

    with tile.TileContext(nc) as tc, ExitStack() as top:
        ent = top.enter_context
        consts = ent(tc.tile_pool(name="consts", bufs=1))
        res_pool = ent(tc.tile_pool(name="res", bufs=1))
        proj_stack = ExitStack()
        w_pool = proj_stack.enter_context(tc.tile_pool(name="w", bufs=2))
        x_pool = proj_stack.enter_context(tc.tile_pool(name="xs", bufs=2))

        ones_sb = consts.tile([128, 128], BF16)
        nc.vector.memset(ones_sb, 1.0)
        tri_sb = consts.tile([128, 128], BF16)
        nc.sync.dma_start(out=tri_sb, in_=tri[:])

        nc.gpsimd.collective_compute(
            "AllGather",
            mybir.AluOpType.bypass,
            ins=[cc_warm_in[:]],
            outs=[cc_warm_out[:]],
            replica_groups=pair_groups,
        )

        q_all = res_pool.tile([128, HG, T], BF16, name="q_all")
        k_all = res_pool.tile([128, HG, T], BF16, name="k_all")
        v_all = res_pool.tile([128, HG, NKB, DV], BF16, name="v_all")

        def weight_tile(pool):
            return pool.tile([128, NDC, HG * 128], BF16, tag="w", name="wtile")

        def weight_slice(w_sb, w_ext, dci):
            nc.sync.dma_start(
                out=w_sb[:, dci, :], in_=w_ext[dci * 128 : (dci + 1) * 128, :]
            )

        wv_sb = weight_tile(w_pool)
        for c in range(4):
            nc.sync.dma_start(
                out=wv_sb[:, 4 * c : 4 * (c + 1), :],
                in_=wv[4 * c * 128 : 4 * (c + 1) * 128, :].rearrange(
                    "(o p) f -> p o f", p=128
                ),
            )
        wk_sb = weight_tile(w_pool)
        wq_sb = None

        def x_stream(src, qc):
            xs = x_pool.tile([128, NDC, QC], BF16, tag="xs")
            nc.scalar.dma_start(
                out=xs,
                in_=src[:, qc * QC : (qc + 1) * QC].rearrange(
                    "(o p) f -> p o f", p=128
                ),
            )
            return xs

        with (
            tc.tile_pool(name="warmps", bufs=1, space="PSUM") as warmps,
            tc.tile_pool(name="ppsum", bufs=4, space="PSUM") as ppsum,
        ):
            wps = warmps.tile([128, 128], F32)
            for i in range(290):
                nc.tensor.matmul(
                    wps, lhsT=ones_sb, rhs=ones_sb,
                    start=(i == 0), stop=(i == 289),
                )

            for qv in range(NQC):
                xv = x_stream(vT, qv)
                for dci in (range(4) if qv == 0 else []):
                    weight_slice(wk_sb, wk, dci)
                for kbs in range(4):
                    kb = 4 * qv + kbs
                    if kbs == 3 and qv < NQC - 1:
                        for dci in range(4 * (qv + 1), 4 * (qv + 2)):
                            weight_slice(wk_sb, wk, dci)
                    for nn in range(2):
                        ps = ppsum.tile([128, 512], F32, tag="pp")
                        for dci in range(NDC):
                            nc.tensor.matmul(
                                ps,
                                lhsT=xv[:, dci, kbs * 128 : (kbs + 1) * 128],
                                rhs=wv_sb[:, dci, nn * 512 : (nn + 1) * 512],
                                start=(dci == 0),
                                stop=(dci == NDC - 1),
                            )
                        nc.vector.tensor_copy(
                            out=v_all[:, nn * 4 : (nn + 1) * 4, kb, :], in_=ps
                        )

            wq_sb = weight_tile(w_pool)
            for qc in range(NQC):
                xs = x_stream(kT, qc)
                for dci in range(4 * qc, 4 * qc + 4):
                    weight_slice(wq_sb, wq, dci)
                for h in range(HG):
                    ps = ppsum.tile([128, QC], F32, tag="pp")
                    for dci in range(NDC):
                        nc.tensor.matmul(
                            ps,
                            lhsT=wk_sb[:, dci, h * 128 : (h + 1) * 128],
                            rhs=xs[:, dci, :],
                            start=(dci == 0),
                            stop=(dci == NDC - 1),
                        )
                    nc.vector.tensor_copy(
                        out=k_all[:, h, qc * QC : (qc + 1) * QC], in_=ps
                    )

            for qc in ATT_ORDER:
                xs = x_stream(qT, qc)
                for h in range(HG):
                    ps = ppsum.tile([128, QC], F32, tag="pp")
                    for dci in range(NDC):
                        nc.tensor.matmul(
                            ps,
                            lhsT=wq_sb[:, dci, h * 128 : (h + 1) * 128],
                            rhs=xs[:, dci, :],
                            start=(dci == 0),
                            stop=(dci == NDC - 1),
                        )
                    nc.vector.tensor_copy(
                        out=q_all[:, h, qc * QC : (qc + 1) * QC], in_=ps
                    )

        proj_stack.close()

        wo_pool = ent(tc.tile_pool(name="wop", bufs=1))
        wo_sb = wo_pool.tile([128, NDC, D // 2], BF16, name="wo_sb")
        for dci in range(NDC):
            weight_slice(wo_sb, wo, dci)
        pt_pool = ent(tc.tile_pool(name="pt", bufs=10 if causal else 6))
        racc_pool = ent(tc.tile_pool(name="racc", bufs=4 if causal else 3))
        mstage = ent(tc.tile_pool(name="mstage", bufs=4 if causal else 3))
        rinv_pool = ent(tc.tile_pool(name="rinv", bufs=3 if causal else 2))
        mfq_pool = ent(tc.tile_pool(name="mfq", bufs=2 if causal else 1))
        ob_pool = ent(tc.tile_pool(name="ob", bufs=4))
        gm_pool = ent(tc.tile_pool(name="gm", bufs=2)) if not causal else None
        spsum = ent(tc.tile_pool(name="spsum", bufs=2, space="PSUM"))
        opsum = ent(tc.tile_pool(name="opsum", bufs=2, space="PSUM"))
        rpsum = ent(tc.tile_pool(name="rpsum", bufs=2, space="PSUM"))
        wpsum = ent(tc.tile_pool(name="wpsum", bufs=2, space="PSUM"))

        def load_gm(qc):
            if causal:
                return None
            gm = gm_pool.tile([128, NKB, QC], BF16, tag="gm")
            nc.scalar.dma_start(
                out=gm,
                in_=maskT[:, qc * QC : (qc + 1) * QC].rearrange(
                    "(o p) f -> p o f", p=128
                ),
            )
            return gm

        def att_head(qc, h, gm):
            nkb = 4 * (qc + 1) if causal else NKB
            ngrp = (nkb + 3) // 4
            grp_order = ([ngrp - 1] + list(range(ngrp - 1))) if causal else \
                list(range(ngrp))
            kb_order = [4 * g + j for g in grp_order for j in range(4)
                        if 4 * g + j < nkb]
            o_ps = opsum.tile([128, QC], F32, tag="opsum")
            r_ps = rpsum.tile([128, QC], F32, tag="rpsum")
            racc = None
            for kbi, kb in enumerate(kb_order):
                j0 = kb_start(qc, kb)
                s_ps = spsum.tile([128, QC], F32, tag="spsum")
                nc.tensor.matmul(
                    s_ps[:, j0:],
                    lhsT=k_all[:, h, kb * 128 : (kb + 1) * 128],
                    rhs=q_all[:, h, qc * QC + j0 : (qc + 1) * QC],
                    start=True,
                    stop=True,
                )
                pt = pt_pool.tile([128, QC], BF16, tag="pt")
                nc.scalar.activation(
                    out=pt[:, j0:],
                    in_=s_ps[:, j0:],
                    func=mybir.ActivationFunctionType.Exp,
                    scale=float(SCALE),
                )
                if causal:
                    if j0 < QC and kb - 4 * qc >= 0:
                        nc.vector.tensor_mul(
                            out=pt[:, j0 : j0 + 128],
                            in0=pt[:, j0 : j0 + 128],
                            in1=tri_sb,
                        )
                else:
                    nc.vector.tensor_mul(out=pt, in0=pt, in1=gm[:, kb, :])
                nc.tensor.matmul(
                    o_ps[:, j0:],
                    lhsT=v_all[:, h, kb, :],
                    rhs=pt[:, j0:],
                    start=(kbi == 0),
                    stop=(kbi == nkb - 1),
                )
                if kbi % 4 == 0:
                    racc = racc_pool.tile([128, QC], BF16, tag="racc")
                    nc.vector.tensor_copy(out=racc, in_=pt)
                else:
                    nc.vector.tensor_add(
                        out=racc[:, j0:], in0=racc[:, j0:], in1=pt[:, j0:]
                    )
                if kbi % 4 == 3 or kbi == nkb - 1:
                    nc.tensor.matmul(
                        r_ps,
                        lhsT=ones_sb,
                        rhs=racc,
                        start=(kbi // 4 == 0),
                        stop=(kbi // 4 == ngrp - 1),
                    )
            rinv = rinv_pool.tile([128, QC], F32, tag="rinv")
            nc.vector.reciprocal_approx_fast(out=rinv, in_=r_ps)
            msb = mstage.tile([128, QC], BF16, tag="mstage")
            nc.vector.tensor_mul(out=msb, in0=o_ps, in1=rinv)
            nc.sync.dma_start(out=cc_in[qc][:, h, :], in_=msb)
            if h == HG - 1:
                nc.gpsimd.collective_compute(
                    "AllGather",
                    mybir.AluOpType.bypass,
                    ins=[cc_in[qc][:]],
                    outs=[cc_out[qc][:]],
                    replica_groups=pair_groups,
                )

        def wo_load(qc):
            mfq = mfq_pool.tile([128, H, QC], BF16, tag="mfq")
            nc.sync.dma_start(out=mfq[:, :HG, :], in_=cc_out[qc][0])
            nc.sync.dma_start(out=mfq[:, HG:, :], in_=cc_out[qc][1])
            return mfq

        def wo_col(qc, col, mfq):
            w_ps = wpsum.tile([128, QC], F32, tag="wpsum")
            for hv in range(H):
                nc.tensor.matmul(
                    w_ps,
                    lhsT=wo_sb[:, hv, col * 128 : (col + 1) * 128],
                    rhs=mfq[:, hv, :],
                    start=(hv == 0),
                    stop=(hv == H - 1),
                )
            ob = ob_pool.tile([128, QC], F32, tag="ob")
            nc.scalar.activation(
                out=ob, in_=w_ps, func=mybir.ActivationFunctionType.Copy
            )
            nc.scalar.dma_start(
                out=outT[
                    col * 128 : (col + 1) * 128,
                    qc * QC : (qc + 1) * QC,
                ],
                in_=ob,
            )

        gm3 = load_gm(3)
        for h in range(HG):
            att_head(3, h, gm3)
        gm0 = load_gm(0)
        for h in range(HG):
            att_head(0, h, gm0)
        mfq3 = wo_load(3)
        gm2 = load_gm(2)
        for h in range(HG):
            att_head(2, h, gm2)
        mfq0 = wo_load(0)
        gm1 = load_gm(1)
        for h in range(HG):
            att_head(1, h, gm1)
            wo_col(3, h, mfq3)
        mfq2 = wo_load(2)
        for col in range(NCOL):
            wo_col(0, col, mfq0)
        mfq1 = wo_load(1)
        for col in range(NCOL):
            wo_col(2, col, mfq2)
        for col in range(NCOL):
            wo_col(1, col, mfq1)

    nc.compile()
    return nc


def kernel(q, k, v, mask, Wq, Wk, Wv, Wo):
    q = np.asarray(q)
    k = np.asarray(k)
    v = np.asarray(v)
    mask = np.asarray(mask)
    causal = bool(np.array_equal(mask, np.tril(np.ones((T, T), dtype=bool))))

    if causal not in _KERNEL_CACHE:
        _KERNEL_CACHE[causal] = (
            build_kernel_causal() if causal else build_kernel_legacy(False)
        )
    nc = _KERNEL_CACHE[causal]

    bf = ml_dtypes.bfloat16
    Wq_b = np.asarray(Wq).astype(bf)
    Wk_b = np.asarray(Wk).astype(bf)
    Wv_b = np.asarray(Wv).astype(bf)
    Wo_b = np.asarray(Wo).astype(bf)
    i = np.arange(128)
    tri_np = (i[None, :] >= i[:, None]).astype(bf)  # tri[k, j] = j >= k
    maskT_np = None if causal else np.ascontiguousarray(mask.T).astype(bf)

    in_maps = []
    for c in range(N_CORES):
        b, g = c // 2, c % 2
        m = {
            "qT": np.ascontiguousarray(q[b].T).astype(bf),
            "kT": np.ascontiguousarray(k[b].T).astype(bf),
            "vT": np.ascontiguousarray(v[b].T).astype(bf),
            "wq": np.ascontiguousarray(Wq_b[:, g * 1024 : (g + 1) * 1024]),
            "wk": np.ascontiguousarray(Wk_b[:, g * 1024 : (g + 1) * 1024]),
            "wv": np.ascontiguousarray(Wv_b[:, g * 1024 : (g + 1) * 1024]),
            "wo": np.ascontiguousarray(Wo_b[:, g * 1024 : (g + 1) * 1024]),
            "tri": tri_np,
        }
        if not causal:
            m["maskT"] = maskT_np
        in_maps.append(m)

    trace = bool(os.environ.get("BASS_KERNEL_TRACE")) and (
        "antenv.axon_hooks" in sys.modules
    )
    res = run_bass_kernel_spmd(nc, in_maps, list(range(N_CORES)), trace=trace)
    if trace and res.exec_time_ns is not None:
        print(f"HW exec time: {res.exec_time_ns} ns")
        kernel.last_exec_time_ns = res.exec_time_ns
        kernel.last_results = res

    out = np.empty((B, T, D), dtype=np.float32)
    for b in range(B):
        top = res.results[2 * b]["outT"]        # cols 0..1023, [1024, 2048]
        bot = res.results[2 * b + 1]["outT"]    # cols 1024..2047
        out[b] = np.concatenate([top, bot], axis=0).T
    return out


# revision 33
# speedup vs baseline: 1.1921x; 1.1871x over previous
"""Distributed multi-head causal attention for 8 TRN2 NeuronCores.

Problem: B=4, T=2048, D=2048, H=16 heads of dk=dv=128.
  out = softmax(mask((q@Wq)(k@Wk)^T / sqrt(dk))) @ (v@Wv) @ Wo

Sharding (2D; all per-core asymmetry lives in host-supplied data so the
SPMD graph is identical on every core):
  core c -> batch b = c//2, head-group g = c%2 (heads 8g..8g+7).
  - QKV projections + attention for (batch b, its 8 heads): fully local.
  - Pair AllGather (replica groups [2b, 2b+1]) exchanges the per-head
    attention outputs (merged^T, bf16) per q-chunk.
  - Output projection: each core computes out^T for its batch for HALF
    the output columns (even core: cols 0..1023, odd: 1024..2047).
  Host reassembles: out[b] = concat(outT_2b, outT_2b+1, axis=0).T

Performance structure (v4):
  - All intermediates (Q^T/K^T per head, V natural) stay RESIDENT IN
    SBUF - no DRAM round trip, no attention-phase input DMAs.
  - Attention chunks run IN ORDER (0,1,2,3). The per-block s->exp->o
    latency bubbles of the in-order PE queue are absorbed by a FILLER
    stream of independent matmuls interleaved between attention
    matmuls: the deferred Q2-tail heads (xs stream still SBUF-resident,
    zero landing latency - fills att0), then the deferred Q3 projection
    (half-chunk xq streams drawn from x_pool slots that free early),
    then the wo output projections of already-gathered chunks, ending
    in a dense wo tail whose runway covers the last exchanges.
  - Pair-exchange granularity matches the CC stream's serial cost
    (~25us/MB + ~3-10us fixed per op): chunks 0/1 gather whole (latency
    hidden), chunk 2 in halves, chunk 3 in head-pairs so the last
    256KB piece lands right behind att3 for the wo3 tail.
  - wv is spread over the sync/gpsimd DGE rings (and xv0 leads the
    scalar ring) so the first V-proj accumulation (which needs all 16
    k-slices of wv) never stalls on ring bootstrap; N_WARM dummy
    matmuls cover that window and keep the PE HAM clock gate at K=8/8.
  - mfq loads ride the otherwise-idle gpsimd ring and outT drains ride
    sync, keeping the scalar engine queue free for the exp pipeline.

Compute is bf16 on TensorE with f32 PSUM accumulation. Softmax skips the
max-subtraction (scores are ~N(0,1); exp is safe in f32) and obtains the
denominators with a ones-matmul per 8 exp-blocks (DVE-accumulated bf16
partial sums); causal masking multiplies exp(scores) by a 0/1 triangular
tile on the single diagonal-crossing 128x128 sub-block, and the moving
free dim of diagonal-region matmuls is trimmed to the unmasked columns.
"""
import os
import sys
from contextlib import ExitStack

import numpy as np
import ml_dtypes

import concourse.bass as bass
import concourse.mybir as mybir
import concourse.tile as tile
from concourse import bacc
from concourse.bass_utils import run_bass_kernel_spmd

BF16 = mybir.dt.bfloat16
F32 = mybir.dt.float32

B, T, D = 4, 2048, 2048
H, DK, DV = 16, 128, 128
HG = 8                      # heads per core
N_CORES = 8
QC = 512                    # q-chunk (matmul moving free dim)
NQC = T // QC               # 4
NKB = T // 128              # 16 k-blocks
NDC = D // 128              # 16 contraction chunks
NCOL = D // 2 // 128        # 8 output-projection column blocks per core
SCALE = 1.0 / np.sqrt(DK)
N_WARM = 300                # dummy matmuls to warm the PE clock gate

_KERNEL_CACHE = {}


def build_kernel_causal():
    nc = bacc.Bacc("TRN2", num_devices=N_CORES)

    qT = nc.declare_dram_parameter("qT", [D, T], BF16, isOutput=False)
    kT = nc.declare_dram_parameter("kT", [D, T], BF16, isOutput=False)
    vT = nc.declare_dram_parameter("vT", [D, T], BF16, isOutput=False)
    wq = nc.declare_dram_parameter("wq", [D, HG * DK], BF16, isOutput=False)
    wk = nc.declare_dram_parameter("wk", [D, HG * DK], BF16, isOutput=False)
    wv = nc.declare_dram_parameter("wv", [D, HG * DV], BF16, isOutput=False)
    wo = nc.declare_dram_parameter("wo", [H * DV, D // 2], BF16, isOutput=False)
    tri = nc.declare_dram_parameter("tri", [128, 128], BF16, isOutput=False)
    outT = nc.declare_dram_parameter("outT", [D // 2, T], F32, isOutput=True)

    # Collective staging (collectives require DRAM in/out). Chunks 0-2:
    # ONE pair AllGather per q-chunk (in [128, HG, QC]) - their ~30us
    # fire-to-land latency is hidden by the schedule. Chunk 3 is split
    # into per-HEAD-PAIR gathers (4 x 256KB, fired after h1/h3/h5/h7):
    # the last one lands ~10us after the last head instead of ~33us,
    # which is what the wo3 tail actually waits on. (Fully per-head
    # gathers were tried and are CC-stream throughput-bound: ~6.7us
    # fixed stream occupancy per op x 32 ops backlogs the stream.)
    cc_in = {qc: nc.dram_tensor(f"cc_in_{qc}", [128, HG, QC], BF16)
             for qc in range(2)}
    cc_out = {qc: nc.dram_tensor(f"cc_out_{qc}", [2, 128, HG, QC], BF16)
              for qc in range(2)}
    cc2_in = {j: nc.dram_tensor(f"cc2_in_{j}", [128, 4, QC], BF16)
              for j in range(2)}
    cc2_out = {j: nc.dram_tensor(f"cc2_out_{j}", [2, 128, 4, QC], BF16)
               for j in range(2)}
    # chunk 3 exchange pieces: pairs (h0-1, h2-3, h4-5) then SINGLES
    # (h6, h7) - each CC op has a ~10-12us serial floor, so the last
    # pieces must be small AND the stream must be clear when they fire.
    cc3_parts = [(0, 2), (2, 2), (4, 2), (6, 1), (7, 1)]
    cc3_in = {j: nc.dram_tensor(f"cc3_in_{j}", [128, n, QC], BF16)
              for j, (h0, n) in enumerate(cc3_parts)}
    cc3_out = {j: nc.dram_tensor(f"cc3_out_{j}", [2, 128, n, QC], BF16)
               for j, (h0, n) in enumerate(cc3_parts)}
    cc_warm_in = nc.dram_tensor("cc_warm_in", [128, 8], BF16)
    cc_warm_out = nc.dram_tensor("cc_warm_out", [2, 128, 8], BF16)
    pair_groups = [[0, 1], [2, 3], [4, 5], [6, 7]]

    def kb_start(qc, kb):
        """First unmasked q column (within the chunk) for this k-block."""
        return min(max((kb - 4 * qc) * 128, 0), QC)

    with tile.TileContext(nc) as tc, ExitStack() as top:
        ent = top.enter_context
        # Pool releases must be LIFO (stack allocator), so creation
        # order is release-reverse: live-to-end pools first (consts,
        # res, B, the attention pools), then A (wv->wq, dies at
        # Q3-drain), then the phase-1 x streams (die at phase-1 end).
        # xq / mfq+ob are pushed later at the then-top of the stack.
        consts = ent(tc.tile_pool(name="consts", bufs=1))
        # SBUF-resident per-head projections (live for the whole kernel).
        res_pool = ent(tc.tile_pool(name="res", bufs=1))
        # Weight zones: A carries wv then (reused) wq; B carries wk then
        # (reused) wo. A closes mid-phase-2 (after the deferred Q3 proj
        # is done); B lives to the end.
        pool_B = ent(tc.tile_pool(name="wB", bufs=1))
        pt_pool = ent(tc.tile_pool(name="pt", bufs=5))
        racc_pool = ent(tc.tile_pool(name="racc", bufs=2))
        mstage = ent(tc.tile_pool(name="mstage", bufs=2))
        rinv_pool = ent(tc.tile_pool(name="rinv", bufs=2))
        stack_A = ExitStack()
        pool_A = stack_A.enter_context(tc.tile_pool(name="wA", bufs=1))
        # Phase-1-only: double-buffered activation streams.
        stack_X = ExitStack()
        x_pool = stack_X.enter_context(tc.tile_pool(name="xs", bufs=2))

        ones_sb = consts.tile([128, 128], BF16)
        nc.vector.memset(ones_sb, 1.0)
        tri_sb = consts.tile([128, 128], BF16)
        nc.sync.dma_start(out=tri_sb, in_=tri[:])

        # Warm the CC stream: the first collective of a NEFF pays the
        # stream barrier + cold-start (~10-30us extra). Fire it now so
        # that happens under the projection phase.
        nc.gpsimd.collective_compute(
            "AllGather",
            mybir.AluOpType.bypass,
            ins=[cc_warm_in[:]],
            outs=[cc_warm_out[:]],
            replica_groups=pair_groups,
        )

        q_all = res_pool.tile([128, HG, T], BF16, name="q_all")
        k_all = res_pool.tile([128, HG, T], BF16, name="k_all")
        v_all = res_pool.tile([128, HG, NKB, DV], BF16, name="v_all")

        def weight_tile(pool):
            return pool.tile([128, NDC, HG * 128], BF16, tag="w", name="wtile")

        def weight_slice(w_sb, w_ext, dci, eng=None):
            (eng or nc.sync).dma_start(
                out=w_sb[:, dci, :], in_=w_ext[dci * 128 : (dci + 1) * 128, :]
            )

        # wv is the startup-critical load: the first V-proj accumulation
        # needs all 16 k-slices, and every DMA ring takes ~10-15us to
        # bootstrap at kernel start. Spread the four 1MB chunks across
        # the THREE DGE rings (sync x2 / gpsimd / scalar-behind-xv0) so
        # they all land by ~20us, inside the warmup window.
        wv_sb = weight_tile(pool_A)
        for c, eng in ((0, nc.sync), (1, nc.gpsimd), (2, nc.gpsimd),
                       (3, nc.sync)):
            eng.dma_start(
                out=wv_sb[:, 4 * c : 4 * (c + 1), :],
                in_=wv[4 * c * 128 : 4 * (c + 1) * 128, :].rearrange(
                    "(o p) f -> p o f", p=128
                ),
            )

        def x_stream(src, qc):
            """[128, NDC, QC] slice of an x^T input, contraction on
            partitions, via the ACT HWDGE ring."""
            xs = x_pool.tile([128, NDC, QC], BF16, tag="xs")
            nc.scalar.dma_start(
                out=xs,
                in_=src[:, qc * QC : (qc + 1) * QC].rearrange(
                    "(o p) f -> p o f", p=128
                ),
            )
            return xs

        xv0 = x_stream(vT, 0)
        wk_sb = weight_tile(pool_B)
        wq_sb = None  # allocated after V proj (reuses wv zone)

        # ------------- Phase 1: V, K, Q0-Q2 projections -------------
        with (
            tc.tile_pool(name="warmps", bufs=1, space="PSUM") as warmps,
            tc.tile_pool(name="ppsum", bufs=4, space="PSUM") as ppsum,
        ):
            # Warm the PE HAM clock gate while the first input DMAs land.
            wps = warmps.tile([128, 128], F32)
            for i in range(N_WARM):
                nc.tensor.matmul(
                    wps, lhsT=ones_sb, rhs=ones_sb,
                    start=(i == 0), stop=(i == N_WARM - 1),
                )

            # V natural ([krows, dv], krows on partitions): stationary is
            # the x^T block, the weight columns stream.
            for qv in range(NQC):
                xv = xv0 if qv == 0 else x_stream(vT, qv)
                for dci in (range(4) if qv == 0 else []):
                    weight_slice(wk_sb, wk, dci)
                for kbs in range(4):
                    kb = 4 * qv + kbs
                    if kbs == 3 and qv < NQC - 1:
                        for dci in range(4 * (qv + 1), 4 * (qv + 2)):
                            weight_slice(wk_sb, wk, dci)
                    for nn in range(2):
                        ps = ppsum.tile([128, 512], F32, tag="pp")
                        for dci in range(NDC):
                            nc.tensor.matmul(
                                ps,
                                lhsT=xv[:, dci, kbs * 128 : (kbs + 1) * 128],
                                rhs=wv_sb[:, dci, nn * 512 : (nn + 1) * 512],
                                start=(dci == 0),
                                stop=(dci == NDC - 1),
                            )
                        nc.vector.tensor_copy(
                            out=v_all[:, nn * 4 : (nn + 1) * 4, kb, :], in_=ps
                        )

            # K^T per head ([dk, q]): weight slice stationary, x^T streams.
            wq_sb = weight_tile(pool_A)  # reuses the wv zone
            for qc in range(NQC):
                xs = x_stream(kT, qc)
                for dci in range(4 * qc, 4 * qc + 4):
                    weight_slice(wq_sb, wq, dci)
                for h in range(HG):
                    ps = ppsum.tile([128, QC], F32, tag="pp")
                    for dci in range(NDC):
                        nc.tensor.matmul(
                            ps,
                            lhsT=wk_sb[:, dci, h * 128 : (h + 1) * 128],
                            rhs=xs[:, dci, :],
                            start=(dci == 0),
                            stop=(dci == NDC - 1),
                        )
                    nc.vector.tensor_copy(
                        out=k_all[:, h, qc * QC : (qc + 1) * QC], in_=ps
                    )

            # Q projection: chunks 0-1 in full, chunk 2 heads 0-3. The
            # rest (Q2 heads 4-7 with the still-resident xs stream, then
            # all of Q3 via fresh xq streams) is deferred into the
            # attention phase as PE filler - Q2-tail fills have ZERO
            # landing latency, which is what att0 needs.
            xs_q2 = None
            for qc in range(3):
                xs = x_stream(qT, qc)
                if qc == 2:
                    xs_q2 = xs
                for h in range(HG if qc < 2 else 4):
                    ps = ppsum.tile([128, QC], F32, tag="pp")
                    for dci in range(NDC):
                        nc.tensor.matmul(
                            ps,
                            lhsT=wq_sb[:, dci, h * 128 : (h + 1) * 128],
                            rhs=xs[:, dci, :],
                            start=(dci == 0),
                            stop=(dci == NDC - 1),
                        )
                    nc.vector.tensor_copy(
                        out=q_all[:, h, qc * QC : (qc + 1) * QC], in_=ps
                    )

        # Deferred-Q3 half-chunk streams: [128, NDC, 256] tiles drawn
        # from x_pool itself (they fit the existing "xs" slots). Slot
        # rotation makes half 0 land during Q2-proj (its slot freed at
        # Q1-proj end) and half 1 land right after the Q2-tail fills
        # release xs_q2's slot - so the Q3 filler never stalls the PE.
        def xq_stream(half):
            xs = x_pool.tile([128, NDC, QC // 2], BF16, tag="xs", name="xqs")
            (nc.scalar if half == 0 else nc.sync).dma_start(
                out=xs,
                in_=qT[:, 3 * QC + half * 256 : 3 * QC + (half + 1) * 256]
                .rearrange("(o p) f -> p o f", p=128),
            )
            return xs

        xq_tiles = [xq_stream(0), xq_stream(1)]

        # wo lands in the SBUF recycled from the wk zone; its 4MB DMA
        # (scalar ring, behind the xq prefetches) waits for the last wk
        # read (end of K proj) and runs under Q0-Q2/att0; first use is
        # the wo0 filler during att2.
        wo_sb = pool_B.tile([128, NDC, D // 2], BF16, tag="w", name="wo_sb")
        for dci in range(NDC):
            weight_slice(wo_sb, wo, dci, eng=nc.scalar)

        # ---------- Phase 2: attention with filler interleave ----------
        spsum = ent(tc.tile_pool(name="spsum", bufs=2, space="PSUM"))
        opsum = ent(tc.tile_pool(name="opsum", bufs=2, space="PSUM"))
        rpsum = ent(tc.tile_pool(name="rpsum", bufs=2, space="PSUM"))
        fill_ps = ent(tc.tile_pool(name="fillps", bufs=2, space="PSUM"))

        stack_M = ExitStack()  # mfq + ob pools, opened after stack_A closes
        mfq_pool_box = {}

        def wo_load(qc):
            # Prefetch the gathered heads for wo(qc).
            mfq = mfq_pool_box["pool"].tile([128, H, QC], BF16, tag="mfq",
                                            name="mfq")
            if qc < 2:
                nc.gpsimd.dma_start(out=mfq[:, :HG, :], in_=cc_out[qc][0])
                nc.gpsimd.dma_start(out=mfq[:, HG:, :], in_=cc_out[qc][1])
            elif qc == 2:
                for j in range(2):
                    nc.gpsimd.dma_start(
                        out=mfq[:, 4 * j : 4 * j + 4, :], in_=cc2_out[j][0]
                    )
                    nc.gpsimd.dma_start(
                        out=mfq[:, HG + 4 * j : HG + 4 * j + 4, :],
                        in_=cc2_out[j][1],
                    )
            else:
                for j, (h0, n) in enumerate(cc3_parts):
                    nc.gpsimd.dma_start(
                        out=mfq[:, h0 : h0 + n, :], in_=cc3_out[j][0]
                    )
                    nc.gpsimd.dma_start(
                        out=mfq[:, HG + h0 : HG + h0 + n, :],
                        in_=cc3_out[j][1],
                    )
            return mfq

        def wo_col_gen(qc, col, mfq):
            w_ps = fill_ps.tile([128, QC], F32, tag="fill")
            for hv in range(H):
                nc.tensor.matmul(
                    w_ps,
                    lhsT=wo_sb[:, hv, col * 128 : (col + 1) * 128],
                    rhs=mfq[:, hv, :],
                    start=(hv == 0),
                    stop=(hv == H - 1),
                )
                yield QC
            # drain on DVE and ship outT on the SYNC ring: the scalar
            # engine queue is the exp pipeline - its DMA_DIRECT2D slots
            # (~0.7us each) would add latency to every exp.
            ob = mfq_pool_box["ob"].tile([128, QC], F32, tag="ob", name="ob")
            nc.vector.tensor_copy(out=ob, in_=w_ps)
            # wo3's drains alternate rings: the ACT queue is exp-free by
            # then and splitting halves the end-of-kernel DMA drain.
            eng = nc.scalar if (qc == 3 and col % 2 == 1) else nc.sync
            eng.dma_start(
                out=outT[
                    col * 128 : (col + 1) * 128,
                    qc * QC : (qc + 1) * QC,
                ],
                in_=ob,
            )

        progress = {"q3": False}

        def filler_gen():
            # -- deferred Q2 heads 4-7: their xs stream is still
            # SBUF-resident, so these fills have no landing latency --
            for h in range(4, HG):
                ps = fill_ps.tile([128, QC], F32, tag="fill")
                for dci in range(NDC):
                    nc.tensor.matmul(
                        ps,
                        lhsT=wq_sb[:, dci, h * 128 : (h + 1) * 128],
                        rhs=xs_q2[:, dci, :],
                        start=(dci == 0),
                        stop=(dci == NDC - 1),
                    )
                    yield QC
                nc.vector.tensor_copy(
                    out=q_all[:, h, 2 * QC : 3 * QC], in_=ps
                )
            # -- deferred Q3 projection, in half-chunks of 256 --
            for half in (0, 1):
                xs = xq_tiles[half]
                for h in range(HG):
                    ps = fill_ps.tile([128, QC], F32, tag="fill")
                    for dci in range(NDC):
                        nc.tensor.matmul(
                            ps[:, : QC // 2],
                            lhsT=wq_sb[:, dci, h * 128 : (h + 1) * 128],
                            rhs=xs[:, dci, :],
                            start=(dci == 0),
                            stop=(dci == NDC - 1),
                        )
                        yield QC // 2
                    nc.vector.tensor_copy(
                        out=q_all[
                            :, h,
                            3 * QC + half * 256 : 3 * QC + (half + 1) * 256,
                        ],
                        in_=ps[:, : QC // 2],
                    )
            progress["q3"] = True
            # -- transition: free the x/wq SBUF, open the mfq + ob pools --
            stack_X.close()
            stack_A.close()
            mfq_pool_box["pool"] = stack_M.enter_context(
                tc.tile_pool(name="mfq", bufs=2)
            )
            mfq_pool_box["ob"] = stack_M.enter_context(
                tc.tile_pool(name="ob", bufs=2)
            )
            # -- wo chunks in gather order; the wo0/1/2 columns left
            # after att3 are the runway that covers the last chunk-3
            # pair exchanges --
            for qc in range(NQC):
                mfq = wo_load(qc)
                for col in range(NCOL):
                    yield from wo_col_gen(qc, col, mfq)

        filler = filler_gen()
        fstate = {"done": False}

        def fill(rows):
            while rows > 0 and not fstate["done"]:
                r = next(filler, None)
                if r is None:
                    fstate["done"] = True
                    return
                rows -= r

        def flush_q3():
            while not progress["q3"] and not fstate["done"]:
                fill(QC)

        def att_head(qc, h):
            filling = True
            # att0-att2 showed 82-94% PE feed at 1-block fills, so fill
            # harder there; att3 keeps a lighter rate so the wo tail
            # retains enough runway to cover the last per-head gathers.
            if qc == 0:
                f_full, f_thin = 768, 1024
            elif qc < 3:
                f_full, f_thin = QC, 768
            else:
                f_full, f_thin = 256, QC
            nkb = 4 * (qc + 1)
            ngrp = (nkb + 3) // 4
            # Process the diagonal-crossing k-group FIRST: its thin
            # (128..512-wide) s->exp->o chains then overlap the dense
            # full-width blocks instead of bunching at the head's end.
            # PSUM accumulation is order-independent; the group-first
            # block always has j0=0 either way.
            grp_order = [ngrp - 1] + list(range(ngrp - 1))
            kb_order = [4 * g + j for g in grp_order for j in range(4)
                        if 4 * g + j < nkb]
            o_ps = opsum.tile([128, QC], F32, tag="opsum")
            r_ps = rpsum.tile([128, QC], F32, tag="rpsum")
            racc = None
            for kbi, kb in enumerate(kb_order):
                j0 = kb_start(qc, kb)  # first live q col in chunk
                s_ps = spsum.tile([128, QC], F32, tag="spsum")
                nc.tensor.matmul(
                    s_ps[:, j0:],
                    lhsT=k_all[:, h, kb * 128 : (kb + 1) * 128],
                    rhs=q_all[:, h, qc * QC + j0 : (qc + 1) * QC],
                    start=True,
                    stop=True,
                )
                # Filler between the s matmul and its dependent o matmul
                # absorbs the exp round-trip latency; thin diagonal
                # blocks leave a bigger bubble, so fill more.
                if filling:
                    fill(f_full if j0 == 0 else f_thin)
                pt = pt_pool.tile([128, QC], BF16, tag="pt")
                nc.scalar.activation(
                    out=pt[:, j0:],
                    in_=s_ps[:, j0:],
                    func=mybir.ActivationFunctionType.Exp,
                    scale=float(SCALE),
                )
                if j0 < QC and 0 <= kb - 4 * qc:
                    # mask the diagonal-crossing 128 columns
                    nc.vector.tensor_mul(
                        out=pt[:, j0 : j0 + 128],
                        in0=pt[:, j0 : j0 + 128],
                        in1=tri_sb,
                    )
                nc.tensor.matmul(
                    o_ps[:, j0:],
                    lhsT=v_all[:, h, kb, :],
                    rhs=pt[:, j0:],
                    start=(kbi == 0),
                    stop=(kbi == nkb - 1),
                )
                # Denominators: sum ALL exp-blocks of the chunk on DVE
                # (bf16), then ONE full-width ones-matmul per head-chunk.
                # The first block in kb_order always has j0=0, so racc is
                # fully initialized.
                if kbi == 0:
                    racc = racc_pool.tile([128, QC], BF16, tag="racc")
                    nc.vector.tensor_copy(out=racc, in_=pt)
                else:
                    nc.vector.tensor_add(
                        out=racc[:, j0:], in0=racc[:, j0:], in1=pt[:, j0:]
                    )
                if kbi == nkb - 1:
                    nc.tensor.matmul(
                        r_ps, lhsT=ones_sb, rhs=racc, start=True, stop=True,
                    )
            # 1/r on DVE: single-instruction NR-seeded approximation
            # (~51 ULP) - the exact reciprocal held the rpsum bank
            # hostage and stalled the PE.
            rinv = rinv_pool.tile([128, QC], F32, tag="rinv")
            nc.vector.reciprocal_approx_fast(out=rinv, in_=r_ps)
            msb = mstage.tile([128, QC], BF16, tag="mstage")
            nc.vector.tensor_mul(out=msb, in0=o_ps, in1=rinv)
            # Exchange granularity is chosen so the CC stream (which
            # serializes ops at ~25us/MB + ~3us fixed) finishes the
            # last chunk-3 piece right behind att3: chunks 0/1 whole
            # (latency hidden), chunk 2 in halves, chunk 3 in pairs.
            if qc < 2:
                nc.sync.dma_start(out=cc_in[qc][:, h, :], in_=msb)
                if h == HG - 1:
                    nc.gpsimd.collective_compute(
                        "AllGather",
                        mybir.AluOpType.bypass,
                        ins=[cc_in[qc][:]],
                        outs=[cc_out[qc][:]],
                        replica_groups=pair_groups,
                    )
            elif qc == 2:
                nc.sync.dma_start(out=cc2_in[h // 4][:, h % 4, :], in_=msb)
                if h % 4 == 3:
                    nc.gpsimd.collective_compute(
                        "AllGather",
                        mybir.AluOpType.bypass,
                        ins=[cc2_in[h // 4][:]],
                        outs=[cc2_out[h // 4][:]],
                        replica_groups=pair_groups,
                    )
            else:
                j = min(h // 2, 2) if h < 6 else h - 3
                h0, n = cc3_parts[j]
                nc.sync.dma_start(out=cc3_in[j][:, h - h0, :], in_=msb)
                if h == h0 + n - 1:
                    nc.gpsimd.collective_compute(
                        "AllGather",
                        mybir.AluOpType.bypass,
                        ins=[cc3_in[j][:]],
                        outs=[cc3_out[j][:]],
                        replica_groups=pair_groups,
                    )

        for qc in range(NQC):
            if qc == 3:
                flush_q3()
            for h in range(HG):
                att_head(qc, h)
        # drain whatever filler remains (dense wo tail)
        while not fstate["done"]:
            fill(QC * 16)
        stack_M.close()

    nc.compile()
    return nc


def build_kernel_legacy(causal: bool):
    """Baseline schedule (kept for the non-causal mask fallback)."""
    nc = bacc.Bacc("TRN2", num_devices=N_CORES)

    qT = nc.declare_dram_parameter("qT", [D, T], BF16, isOutput=False)
    kT = nc.declare_dram_parameter("kT", [D, T], BF16, isOutput=False)
    vT = nc.declare_dram_parameter("vT", [D, T], BF16, isOutput=False)
    wq = nc.declare_dram_parameter("wq", [D, HG * DK], BF16, isOutput=False)
    wk = nc.declare_dram_parameter("wk", [D, HG * DK], BF16, isOutput=False)
    wv = nc.declare_dram_parameter("wv", [D, HG * DV], BF16, isOutput=False)
    wo = nc.declare_dram_parameter("wo", [H * DV, D // 2], BF16, isOutput=False)
    tri = nc.declare_dram_parameter("tri", [128, 128], BF16, isOutput=False)
    if not causal:
        maskT = nc.declare_dram_parameter("maskT", [T, T], BF16, isOutput=False)
    outT = nc.declare_dram_parameter("outT", [D // 2, T], F32, isOutput=True)

    cc_in = {qc: nc.dram_tensor(f"cc_in_{qc}", [128, HG, QC], BF16)
             for qc in range(NQC)}
    cc_out = {qc: nc.dram_tensor(f"cc_out_{qc}", [2, 128, HG, QC], BF16)
              for qc in range(NQC)}
    cc_warm_in = nc.dram_tensor("cc_warm_in", [128, 8], BF16)
    cc_warm_out = nc.dram_tensor("cc_warm_out", [2, 128, 8], BF16)
    pair_groups = [[0, 1], [2, 3], [4, 5], [6, 7]]

    ATT_ORDER = (3, 0, 2, 1)

    def kb_start(qc, kb):
        if not causal:
            return 0
        return min(max((kb - 4 * qc) * 128, 0), QC)

    with tile.TileContext(nc) as tc, ExitStack() as top:
        ent = top.enter_context
        consts = ent(tc.tile_pool(name="consts", bufs=1))
        res_pool = ent(tc.tile_pool(name="res", bufs=1))
        proj_stack = ExitStack()
        w_pool = proj_stack.enter_context(tc.tile_pool(name="w", bufs=2))
        x_pool = proj_stack.enter_context(tc.tile_pool(name="xs", bufs=2))

        ones_sb = consts.tile([128, 128], BF16)
        nc.vector.memset(ones_sb, 1.0)
        tri_sb = consts.tile([128, 128], BF16)
        nc.sync.dma_start(out=tri_sb, in_=tri[:])

        nc.gpsimd.collective_compute(
            "AllGather",
            mybir.AluOpType.bypass,
            ins=[cc_warm_in[:]],
            outs=[cc_warm_out[:]],
            replica_groups=pair_groups,
        )

        q_all = res_pool.tile([128, HG, T], BF16, name="q_all")
        k_all = res_pool.tile([128, HG, T], BF16, name="k_all")
        v_all = res_pool.tile([128, HG, NKB, DV], BF16, name="v_all")

        def weight_tile(pool):
            return pool.tile([128, NDC, HG * 128], BF16, tag="w", name="wtile")

        def weight_slice(w_sb, w_ext, dci):
            nc.sync.dma_start(
                out=w_sb[:, dci, :], in_=w_ext[dci * 128 : (dci + 1) * 128, :]
            )

        wv_sb = weight_tile(w_pool)
        for c in range(4):
            nc.sync.dma_start(
                out=wv_sb[:, 4 * c : 4 * (c + 1), :],
                in_=wv[4 * c * 128 : 4 * (c + 1) * 128, :].rearrange(
                    "(o p) f -> p o f", p=128
                ),
            )
        wk_sb = weight_tile(w_pool)
        wq_sb = None

        def x_stream(src, qc):
            xs = x_pool.tile([128, NDC, QC], BF16, tag="xs")
            nc.scalar.dma_start(
                out=xs,
                in_=src[:, qc * QC : (qc + 1) * QC].rearrange(
                    "(o p) f -> p o f", p=128
                ),
            )
            return xs

        with (
            tc.tile_pool(name="warmps", bufs=1, space="PSUM") as warmps,
            tc.tile_pool(name="ppsum", bufs=4, space="PSUM") as ppsum,
        ):
            wps = warmps.tile([128, 128], F32)
            for i in range(290):
                nc.tensor.matmul(
                    wps, lhsT=ones_sb, rhs=ones_sb,
                    start=(i == 0), stop=(i == 289),
                )

            for qv in range(NQC):
                xv = x_stream(vT, qv)
                for dci in (range(4) if qv == 0 else []):
                    weight_slice(wk_sb, wk, dci)
                for kbs in range(4):
                    kb = 4 * qv + kbs
                    if kbs == 3 and qv < NQC - 1:
                        for dci in range(4 * (qv + 1), 4 * (qv + 2)):
                            weight_slice(wk_sb, wk, dci)
                    for nn in range(2):
                        ps = ppsum.tile([128, 512], F32, tag="pp")
                        for dci in range(NDC):
                            nc.tensor.matmul(
                                ps,
                                lhsT=xv[:, dci, kbs * 128 : (kbs + 1) * 128],
                                rhs=wv_sb[:, dci, nn * 512 : (nn + 1) * 512],
                                start=(dci == 0),
                                stop=(dci == NDC - 1),
                            )
                        nc.vector.tensor_copy(
                            out=v_all[:, nn * 4 : (nn + 1) * 4, kb, :], in_=ps
                        )

            wq_sb = weight_tile(w_pool)
            for qc in range(NQC):
                xs = x_stream(kT, qc)
                for dci in range(4 * qc, 4 * qc + 4):
                    weight_slice(wq_sb, wq, dci)
                for h in range(HG):
                    ps = ppsum.tile([128, QC], F32, tag="pp")
                    for dci in range(NDC):
                        nc.tensor.matmul(
                            ps,
                            lhsT=wk_sb[:, dci, h * 128 : (h + 1) * 128],
                            rhs=xs[:, dci, :],
                            start=(dci == 0),
                            stop=(dci == NDC - 1),
                        )
                    nc.vector.tensor_copy(
                        out=k_all[:, h, qc * QC : (qc + 1) * QC], in_=ps
                    )

            for qc in ATT_ORDER:
                xs = x_stream(qT, qc)
                for h in range(HG):
                    ps = ppsum.tile([128, QC], F32, tag="pp")
                    for dci in range(NDC):
                        nc.tensor.matmul(
                            ps,
                            lhsT=wq_sb[:, dci, h * 128 : (h + 1) * 128],
                            rhs=xs[:, dci, :],
                            start=(dci == 0),
                            stop=(dci == NDC - 1),
                        )
                    nc.vector.tensor_copy(
                        out=q_all[:, h, qc * QC : (qc + 1) * QC], in_=ps
                    )

        proj_stack.close()

        wo_pool = ent(tc.tile_pool(name="wop", bufs=1))
        wo_sb = wo_pool.tile([128, NDC, D // 2], BF16, name="wo_sb")
        for dci in range(NDC):
            weight_slice(wo_sb, wo, dci)
        pt_pool = ent(tc.tile_pool(name="pt", bufs=10 if causal else 6))
        racc_pool = ent(tc.tile_pool(name="racc", bufs=4 if causal else 3))
        mstage = ent(tc.tile_pool(name="mstage", bufs=4 if causal else 3))
        rinv_pool = ent(tc.tile_pool(name="rinv", bufs=3 if causal else 2))
        mfq_pool = ent(tc.tile_pool(name="mfq", bufs=2 if causal else 1))
        ob_pool = ent(tc.tile_pool(name="ob", bufs=4))
        gm_pool = ent(tc.tile_pool(name="gm", bufs=2)) if not causal else None
        spsum = ent(tc.tile_pool(name="spsum", bufs=2, space="PSUM"))
        opsum = ent(tc.tile_pool(name="opsum", bufs=2, space="PSUM"))
        rpsum = ent(tc.tile_pool(name="rpsum", bufs=2, space="PSUM"))
        wpsum = ent(tc.tile_pool(name="wpsum", bufs=2, space="PSUM"))

        def load_gm(qc):
            if causal:
                return None
            gm = gm_pool.tile([128, NKB, QC], BF16, tag="gm")
            nc.scalar.dma_start(
                out=gm,
                in_=maskT[:, qc * QC : (qc + 1) * QC].rearrange(
                    "(o p) f -> p o f", p=128
                ),
            )
            return gm

        def att_head(qc, h, gm):
            nkb = 4 * (qc + 1) if causal else NKB
            ngrp = (nkb + 3) // 4
            grp_order = ([ngrp - 1] + list(range(ngrp - 1))) if causal else \
                list(range(ngrp))
            kb_order = [4 * g + j for g in grp_order for j in range(4)
                        if 4 * g + j < nkb]
            o_ps = opsum.tile([128, QC], F32, tag="opsum")
            r_ps = rpsum.tile([128, QC], F32, tag="rpsum")
            racc = None
            for kbi, kb in enumerate(kb_order):
                j0 = kb_start(qc, kb)
                s_ps = spsum.tile([128, QC], F32, tag="spsum")
                nc.tensor.matmul(
                    s_ps[:, j0:],
                    lhsT=k_all[:, h, kb * 128 : (kb + 1) * 128],
                    rhs=q_all[:, h, qc * QC + j0 : (qc + 1) * QC],
                    start=True,
                    stop=True,
                )
                pt = pt_pool.tile([128, QC], BF16, tag="pt")
                nc.scalar.activation(
                    out=pt[:, j0:],
                    in_=s_ps[:, j0:],
                    func=mybir.ActivationFunctionType.Exp,
                    scale=float(SCALE),
                )
                if causal:
                    if j0 < QC and kb - 4 * qc >= 0:
                        nc.vector.tensor_mul(
                            out=pt[:, j0 : j0 + 128],
                            in0=pt[:, j0 : j0 + 128],
                            in1=tri_sb,
                        )
                else:
                    nc.vector.tensor_mul(out=pt, in0=pt, in1=gm[:, kb, :])
                nc.tensor.matmul(
                    o_ps[:, j0:],
                    lhsT=v_all[:, h, kb, :],
                    rhs=pt[:, j0:],
                    start=(kbi == 0),
                    stop=(kbi == nkb - 1),
                )
                if kbi % 4 == 0:
                    racc = racc_pool.tile([128, QC], BF16, tag="racc")
                    nc.vector.tensor_copy(out=racc, in_=pt)
                else:
                    nc.vector.tensor_add(
                        out=racc[:, j0:], in0=racc[:, j0:], in1=pt[:, j0:]
                    )
                if kbi % 4 == 3 or kbi == nkb - 1:
                    nc.tensor.matmul(
                        r_ps,
                        lhsT=ones_sb,
                        rhs=racc,
                        start=(kbi // 4 == 0),
                        stop=(kbi // 4 == ngrp - 1),
                    )
            rinv = rinv_pool.tile([128, QC], F32, tag="rinv")
            nc.vector.reciprocal_approx_fast(out=rinv, in_=r_ps)
            msb = mstage.tile([128, QC], BF16, tag="mstage")
            nc.vector.tensor_mul(out=msb, in0=o_ps, in1=rinv)
            nc.sync.dma_start(out=cc_in[qc][:, h, :], in_=msb)
            if h == HG - 1:
                nc.gpsimd.collective_compute(
                    "AllGather",
                    mybir.AluOpType.bypass,
                    ins=[cc_in[qc][:]],
                    outs=[cc_out[qc][:]],
                    replica_groups=pair_groups,
                )

        def wo_load(qc):
            mfq = mfq_pool.tile([128, H, QC], BF16, tag="mfq")
            nc.sync.dma_start(out=mfq[:, :HG, :], in_=cc_out[qc][0])
            nc.sync.dma_start(out=mfq[:, HG:, :], in_=cc_out[qc][1])
            return mfq

        def wo_col(qc, col, mfq):
            w_ps = wpsum.tile([128, QC], F32, tag="wpsum")
            for hv in range(H):
                nc.tensor.matmul(
                    w_ps,
                    lhsT=wo_sb[:, hv, col * 128 : (col + 1) * 128],
                    rhs=mfq[:, hv, :],
                    start=(hv == 0),
                    stop=(hv == H - 1),
                )
            ob = ob_pool.tile([128, QC], F32, tag="ob")
            nc.scalar.activation(
                out=ob, in_=w_ps, func=mybir.ActivationFunctionType.Copy
            )
            nc.scalar.dma_start(
                out=outT[
                    col * 128 : (col + 1) * 128,
                    qc * QC : (qc + 1) * QC,
                ],
                in_=ob,
            )

        gm3 = load_gm(3)
        for h in range(HG):
            att_head(3, h, gm3)
        gm0 = load_gm(0)
        for h in range(HG):
            att_head(0, h, gm0)
        mfq3 = wo_load(3)
        gm2 = load_gm(2)
        for h in range(HG):
            att_head(2, h, gm2)
        mfq0 = wo_load(0)
        gm1 = load_gm(1)
        for h in range(HG):
            att_head(1, h, gm1)
            wo_col(3, h, mfq3)
        mfq2 = wo_load(2)
        for col in range(NCOL):
            wo_col(0, col, mfq0)
        mfq1 = wo_load(1)
        for col in range(NCOL):
            wo_col(2, col, mfq2)
        for col in range(NCOL):
            wo_col(1, col, mfq1)

    nc.compile()
    return nc


def kernel(q, k, v, mask, Wq, Wk, Wv, Wo):
    q = np.asarray(q)
    k = np.asarray(k)
    v = np.asarray(v)
    mask = np.asarray(mask)
    causal = bool(np.array_equal(mask, np.tril(np.ones((T, T), dtype=bool))))

    if causal not in _KERNEL_CACHE:
        _KERNEL_CACHE[causal] = (
            build_kernel_causal() if causal else build_kernel_legacy(False)
        )
    nc = _KERNEL_CACHE[causal]

    bf = ml_dtypes.bfloat16
    Wq_b = np.asarray(Wq).astype(bf)
    Wk_b = np.asarray(Wk).astype(bf)
    Wv_b = np.asarray(Wv).astype(bf)
    Wo_b = np.asarray(Wo).astype(bf)
    i = np.arange(128)
    tri_np = (i[None, :] >= i[:, None]).astype(bf)  # tri[k, j] = j >= k
    maskT_np = None if causal else np.ascontiguousarray(mask.T).astype(bf)

    in_maps = []
    for c in range(N_CORES):
        b, g = c // 2, c % 2
        m = {
            "qT": np.ascontiguousarray(q[b].T).astype(bf),
            "kT": np.ascontiguousarray(k[b].T).astype(bf),
            "vT": np.ascontiguousarray(v[b].T).astype(bf),
            "wq": np.ascontiguousarray(Wq_b[:, g * 1024 : (g + 1) * 1024]),
            "wk": np.ascontiguousarray(Wk_b[:, g * 1024 : (g + 1) * 1024]),
            "wv": np.ascontiguousarray(Wv_b[:, g * 1024 : (g + 1) * 1024]),
            "wo": np.ascontiguousarray(Wo_b[:, g * 1024 : (g + 1) * 1024]),
            "tri": tri_np,
        }
        if not causal:
            m["maskT"] = maskT_np
        in_maps.append(m)

    trace = bool(os.environ.get("BASS_KERNEL_TRACE")) and (
        "antenv.axon_hooks" in sys.modules
    )
    res = run_bass_kernel_spmd(nc, in_maps, list(range(N_CORES)), trace=trace)
    if trace and res.exec_time_ns is not None:
        print(f"HW exec time: {res.exec_time_ns} ns")
        kernel.last_exec_time_ns = res.exec_time_ns
        kernel.last_results = res

    out = np.empty((B, T, D), dtype=np.float32)
    for b in range(B):
        top = res.results[2 * b]["outT"]        # cols 0..1023, [1024, 2048]
        bot = res.results[2 * b + 1]["outT"]    # cols 1024..2047
        out[b] = np.concatenate([top, bot], axis=0).T
    return out


# revision 34
# speedup vs baseline: 1.2087x; 1.0140x over previous
"""Distributed multi-head causal attention for 8 TRN2 NeuronCores.

Problem: B=4, T=2048, D=2048, H=16 heads of dk=dv=128.
  out = softmax(mask((q@Wq)(k@Wk)^T / sqrt(dk))) @ (v@Wv) @ Wo

Sharding (2D; all per-core asymmetry lives in host-supplied data so the
SPMD graph is identical on every core):
  core c -> batch b = c//2, head-group g = c%2 (heads 8g..8g+7).
  - QKV projections + attention for (batch b, its 8 heads): fully local.
  - Pair AllGather (replica groups [2b, 2b+1]) exchanges the per-head
    attention outputs (merged^T, bf16) per q-chunk.
  - Output projection: each core computes out^T for its batch for HALF
    the output columns (even core: cols 0..1023, odd: 1024..2047).
  Host reassembles: out[b] = concat(outT_2b, outT_2b+1, axis=0).T

Performance structure (v4):
  - All intermediates (Q^T/K^T per head, V natural) stay RESIDENT IN
    SBUF - no DRAM round trip, no attention-phase input DMAs.
  - Attention chunks run IN ORDER (0,1,2,3). The per-block s->exp->o
    latency bubbles of the in-order PE queue are absorbed by a FILLER
    stream of independent matmuls interleaved between attention
    matmuls: the deferred Q2-tail heads (xs stream still SBUF-resident,
    zero landing latency - fills att0), then the deferred Q3 projection
    (half-chunk xq streams drawn from x_pool slots that free early),
    then the wo output projections of already-gathered chunks, ending
    in a dense wo tail whose runway covers the last exchanges.
  - Pair-exchange granularity matches the CC stream's serial cost
    (~25us/MB + ~3-10us fixed per op): chunks 0/1 gather whole (latency
    hidden), chunk 2 in halves, chunk 3 in head-pairs so the last
    256KB piece lands right behind att3 for the wo3 tail.
  - wv is spread over the sync/gpsimd DGE rings (and xv0 leads the
    scalar ring) so the first V-proj accumulation (which needs all 16
    k-slices of wv) never stalls on ring bootstrap; N_WARM dummy
    matmuls cover that window and keep the PE HAM clock gate at K=8/8.
  - mfq loads ride the otherwise-idle gpsimd ring and outT drains ride
    sync, keeping the scalar engine queue free for the exp pipeline.

Compute is bf16 on TensorE with f32 PSUM accumulation. Softmax skips the
max-subtraction (scores are ~N(0,1); exp is safe in f32) and obtains the
denominators with a ones-matmul per 8 exp-blocks (DVE-accumulated bf16
partial sums); causal masking multiplies exp(scores) by a 0/1 triangular
tile on the single diagonal-crossing 128x128 sub-block, and the moving
free dim of diagonal-region matmuls is trimmed to the unmasked columns.
"""
import os
import sys
from contextlib import ExitStack

import numpy as np
import ml_dtypes

import concourse.bass as bass
import concourse.mybir as mybir
import concourse.tile as tile
from concourse import bacc
from concourse.bass_utils import run_bass_kernel_spmd

BF16 = mybir.dt.bfloat16
F32 = mybir.dt.float32

B, T, D = 4, 2048, 2048
H, DK, DV = 16, 128, 128
HG = 8                      # heads per core
N_CORES = 8
QC = 512                    # q-chunk (matmul moving free dim)
NQC = T // QC               # 4
NKB = T // 128              # 16 k-blocks
NDC = D // 128              # 16 contraction chunks
NCOL = D // 2 // 128        # 8 output-projection column blocks per core
SCALE = 1.0 / np.sqrt(DK)
N_WARM = 300                # dummy matmuls to warm the PE clock gate

_KERNEL_CACHE = {}


def build_kernel_causal():
    nc = bacc.Bacc("TRN2", num_devices=N_CORES)

    qT = nc.declare_dram_parameter("qT", [D, T], BF16, isOutput=False)
    kT = nc.declare_dram_parameter("kT", [D, T], BF16, isOutput=False)
    vT = nc.declare_dram_parameter("vT", [D, T], BF16, isOutput=False)
    wq = nc.declare_dram_parameter("wq", [D, HG * DK], BF16, isOutput=False)
    wk = nc.declare_dram_parameter("wk", [D, HG * DK], BF16, isOutput=False)
    wv = nc.declare_dram_parameter("wv", [D, HG * DV], BF16, isOutput=False)
    wo = nc.declare_dram_parameter("wo", [H * DV, D // 2], BF16, isOutput=False)
    tri = nc.declare_dram_parameter("tri", [128, 128], BF16, isOutput=False)
    outT = nc.declare_dram_parameter("outT", [D // 2, T], F32, isOutput=True)

    # Collective staging (collectives require DRAM in/out). Chunks 0-2:
    # ONE pair AllGather per q-chunk (in [128, HG, QC]) - their ~30us
    # fire-to-land latency is hidden by the schedule. Chunk 3 is split
    # into per-HEAD-PAIR gathers (4 x 256KB, fired after h1/h3/h5/h7):
    # the last one lands ~10us after the last head instead of ~33us,
    # which is what the wo3 tail actually waits on. (Fully per-head
    # gathers were tried and are CC-stream throughput-bound: ~6.7us
    # fixed stream occupancy per op x 32 ops backlogs the stream.)
    cc_in = {qc: nc.dram_tensor(f"cc_in_{qc}", [128, HG, QC], BF16)
             for qc in range(2)}
    cc_out = {qc: nc.dram_tensor(f"cc_out_{qc}", [2, 128, HG, QC], BF16)
              for qc in range(2)}
    cc2_in = {j: nc.dram_tensor(f"cc2_in_{j}", [128, 4, QC], BF16)
              for j in range(2)}
    cc2_out = {j: nc.dram_tensor(f"cc2_out_{j}", [2, 128, 4, QC], BF16)
               for j in range(2)}
    # chunk 3 exchange pieces: pairs (h0-1, h2-3, h4-5) then SINGLES
    # (h6, h7) - each CC op has a ~10-12us serial floor, so the last
    # pieces must be small AND the stream must be clear when they fire.
    cc3_parts = [(0, 2), (2, 2), (4, 2), (6, 1), (7, 1)]
    cc3_in = {j: nc.dram_tensor(f"cc3_in_{j}", [128, n, QC], BF16)
              for j, (h0, n) in enumerate(cc3_parts)}
    cc3_out = {j: nc.dram_tensor(f"cc3_out_{j}", [2, 128, n, QC], BF16)
               for j, (h0, n) in enumerate(cc3_parts)}
    cc_warm_in = nc.dram_tensor("cc_warm_in", [128, 8], BF16)
    cc_warm_out = nc.dram_tensor("cc_warm_out", [2, 128, 8], BF16)
    pair_groups = [[0, 1], [2, 3], [4, 5], [6, 7]]

    def kb_start(qc, kb):
        """First unmasked q column (within the chunk) for this k-block."""
        return min(max((kb - 4 * qc) * 128, 0), QC)

    with tile.TileContext(nc) as tc, ExitStack() as top:
        ent = top.enter_context
        # Pool releases must be LIFO (stack allocator), so creation
        # order is release-reverse: live-to-end pools first (consts,
        # res, B, the attention pools), then A (wv->wq, dies at
        # Q3-drain), then the phase-1 x streams (die at phase-1 end).
        # xq / mfq+ob are pushed later at the then-top of the stack.
        consts = ent(tc.tile_pool(name="consts", bufs=1))
        # SBUF-resident per-head projections (live for the whole kernel).
        res_pool = ent(tc.tile_pool(name="res", bufs=1))
        # Weight zones: A carries wv then (reused) wq; B carries wk then
        # (reused) wo. A closes mid-phase-2 (after the deferred Q3 proj
        # is done); B lives to the end.
        pool_B = ent(tc.tile_pool(name="wB", bufs=1))
        pt_pool = ent(tc.tile_pool(name="pt", bufs=5))
        racc_pool = ent(tc.tile_pool(name="racc", bufs=2))
        mstage = ent(tc.tile_pool(name="mstage", bufs=2))
        rinv_pool = ent(tc.tile_pool(name="rinv", bufs=2))
        stack_A = ExitStack()
        pool_A = stack_A.enter_context(tc.tile_pool(name="wA", bufs=1))
        # Phase-1-only: double-buffered activation streams.
        stack_X = ExitStack()
        x_pool = stack_X.enter_context(tc.tile_pool(name="xs", bufs=2))

        ones_sb = consts.tile([128, 128], BF16)
        nc.vector.memset(ones_sb, 1.0)
        tri_sb = consts.tile([128, 128], BF16)
        nc.sync.dma_start(out=tri_sb, in_=tri[:])

        # Warm the CC stream: the first collective of a NEFF pays the
        # stream barrier + cold-start (~10-30us extra). Fire it now so
        # that happens under the projection phase.
        nc.gpsimd.collective_compute(
            "AllGather",
            mybir.AluOpType.bypass,
            ins=[cc_warm_in[:]],
            outs=[cc_warm_out[:]],
            replica_groups=pair_groups,
        )

        q_all = res_pool.tile([128, HG, T], BF16, name="q_all")
        k_all = res_pool.tile([128, HG, T], BF16, name="k_all")
        v_all = res_pool.tile([128, HG, NKB, DV], BF16, name="v_all")

        def weight_tile(pool):
            return pool.tile([128, NDC, HG * 128], BF16, tag="w", name="wtile")

        def weight_slice(w_sb, w_ext, dci, eng=None):
            (eng or nc.sync).dma_start(
                out=w_sb[:, dci, :], in_=w_ext[dci * 128 : (dci + 1) * 128, :]
            )

        # wv is the startup-critical load: the first V-proj accumulation
        # needs all 16 k-slices, and every DMA ring takes ~10-15us to
        # bootstrap at kernel start. Spread the four 1MB chunks across
        # the THREE DGE rings (sync x2 / gpsimd / scalar-behind-xv0) so
        # they all land by ~20us, inside the warmup window.
        wv_sb = weight_tile(pool_A)
        for c, eng in ((0, nc.sync), (1, nc.sync), (2, nc.gpsimd),
                       (3, nc.sync)):
            eng.dma_start(
                out=wv_sb[:, 4 * c : 4 * (c + 1), :],
                in_=wv[4 * c * 128 : 4 * (c + 1) * 128, :].rearrange(
                    "(o p) f -> p o f", p=128
                ),
            )

        def x_stream(src, qc):
            """[128, NDC, QC] slice of an x^T input, contraction on
            partitions, via the ACT HWDGE ring."""
            xs = x_pool.tile([128, NDC, QC], BF16, tag="xs")
            nc.scalar.dma_start(
                out=xs,
                in_=src[:, qc * QC : (qc + 1) * QC].rearrange(
                    "(o p) f -> p o f", p=128
                ),
            )
            return xs

        xv0 = x_stream(vT, 0)
        wk_sb = weight_tile(pool_B)
        wq_sb = None  # allocated after V proj (reuses wv zone)

        # ------------- Phase 1: V, K, Q0-Q2 projections -------------
        with (
            tc.tile_pool(name="warmps", bufs=1, space="PSUM") as warmps,
            tc.tile_pool(name="ppsum", bufs=4, space="PSUM") as ppsum,
        ):
            # Warm the PE HAM clock gate while the first input DMAs land.
            wps = warmps.tile([128, 128], F32)
            for i in range(N_WARM):
                nc.tensor.matmul(
                    wps, lhsT=ones_sb, rhs=ones_sb,
                    start=(i == 0), stop=(i == N_WARM - 1),
                )

            # V natural ([krows, dv], krows on partitions): stationary is
            # the x^T block, the weight columns stream.
            for qv in range(NQC):
                xv = xv0 if qv == 0 else x_stream(vT, qv)
                for dci in (range(4) if qv == 0 else []):
                    weight_slice(wk_sb, wk, dci)
                for kbs in range(4):
                    kb = 4 * qv + kbs
                    if kbs == 3 and qv < NQC - 1:
                        for dci in range(4 * (qv + 1), 4 * (qv + 2)):
                            weight_slice(wk_sb, wk, dci)
                    for nn in range(2):
                        ps = ppsum.tile([128, 512], F32, tag="pp")
                        for dci in range(NDC):
                            nc.tensor.matmul(
                                ps,
                                lhsT=xv[:, dci, kbs * 128 : (kbs + 1) * 128],
                                rhs=wv_sb[:, dci, nn * 512 : (nn + 1) * 512],
                                start=(dci == 0),
                                stop=(dci == NDC - 1),
                            )
                        nc.vector.tensor_copy(
                            out=v_all[:, nn * 4 : (nn + 1) * 4, kb, :], in_=ps
                        )

            # K^T per head ([dk, q]): weight slice stationary, x^T streams.
            wq_sb = weight_tile(pool_A)  # reuses the wv zone
            for qc in range(NQC):
                xs = x_stream(kT, qc)
                for dci in range(4 * qc, 4 * qc + 4):
                    weight_slice(wq_sb, wq, dci)
                for h in range(HG):
                    ps = ppsum.tile([128, QC], F32, tag="pp")
                    for dci in range(NDC):
                        nc.tensor.matmul(
                            ps,
                            lhsT=wk_sb[:, dci, h * 128 : (h + 1) * 128],
                            rhs=xs[:, dci, :],
                            start=(dci == 0),
                            stop=(dci == NDC - 1),
                        )
                    nc.vector.tensor_copy(
                        out=k_all[:, h, qc * QC : (qc + 1) * QC], in_=ps
                    )

            # Q projection: chunks 0-1 in full, chunk 2 heads 0-3. The
            # rest (Q2 heads 4-7 with the still-resident xs stream, then
            # all of Q3 via fresh xq streams) is deferred into the
            # attention phase as PE filler - Q2-tail fills have ZERO
            # landing latency, which is what att0 needs.
            xs_q2 = None
            for qc in range(3):
                xs = x_stream(qT, qc)
                if qc == 2:
                    xs_q2 = xs
                for h in range(HG if qc < 2 else 4):
                    ps = ppsum.tile([128, QC], F32, tag="pp")
                    for dci in range(NDC):
                        nc.tensor.matmul(
                            ps,
                            lhsT=wq_sb[:, dci, h * 128 : (h + 1) * 128],
                            rhs=xs[:, dci, :],
                            start=(dci == 0),
                            stop=(dci == NDC - 1),
                        )
                    nc.vector.tensor_copy(
                        out=q_all[:, h, qc * QC : (qc + 1) * QC], in_=ps
                    )

        # Deferred-Q3 half-chunk streams: [128, NDC, 256] tiles drawn
        # from x_pool itself (they fit the existing "xs" slots). Slot
        # rotation makes half 0 land during Q2-proj (its slot freed at
        # Q1-proj end) and half 1 land right after the Q2-tail fills
        # release xs_q2's slot - so the Q3 filler never stalls the PE.
        def xq_stream(half):
            xs = x_pool.tile([128, NDC, QC // 2], BF16, tag="xs", name="xqs")
            (nc.scalar if half == 0 else nc.sync).dma_start(
                out=xs,
                in_=qT[:, 3 * QC + half * 256 : 3 * QC + (half + 1) * 256]
                .rearrange("(o p) f -> p o f", p=128),
            )
            return xs

        xq_tiles = [xq_stream(0), xq_stream(1)]

        # wo lands in the SBUF recycled from the wk zone; its 4MB DMA
        # (scalar ring, behind the xq prefetches) waits for the last wk
        # read (end of K proj) and runs under Q0-Q2/att0; first use is
        # the wo0 filler during att2.
        wo_sb = pool_B.tile([128, NDC, D // 2], BF16, tag="w", name="wo_sb")
        for dci in range(NDC):
            weight_slice(wo_sb, wo, dci, eng=nc.scalar)

        # ---------- Phase 2: attention with filler interleave ----------
        spsum = ent(tc.tile_pool(name="spsum", bufs=2, space="PSUM"))
        opsum = ent(tc.tile_pool(name="opsum", bufs=2, space="PSUM"))
        rpsum = ent(tc.tile_pool(name="rpsum", bufs=2, space="PSUM"))
        fill_ps = ent(tc.tile_pool(name="fillps", bufs=2, space="PSUM"))

        stack_M = ExitStack()  # mfq + ob pools, opened after stack_A closes
        mfq_pool_box = {}

        def wo_load(qc):
            # Prefetch the gathered heads for wo(qc).
            mfq = mfq_pool_box["pool"].tile([128, H, QC], BF16, tag="mfq",
                                            name="mfq")
            if qc < 2:
                nc.gpsimd.dma_start(out=mfq[:, :HG, :], in_=cc_out[qc][0])
                nc.gpsimd.dma_start(out=mfq[:, HG:, :], in_=cc_out[qc][1])
            elif qc == 2:
                for j in range(2):
                    nc.gpsimd.dma_start(
                        out=mfq[:, 4 * j : 4 * j + 4, :], in_=cc2_out[j][0]
                    )
                    nc.gpsimd.dma_start(
                        out=mfq[:, HG + 4 * j : HG + 4 * j + 4, :],
                        in_=cc2_out[j][1],
                    )
            else:
                for j, (h0, n) in enumerate(cc3_parts):
                    nc.gpsimd.dma_start(
                        out=mfq[:, h0 : h0 + n, :], in_=cc3_out[j][0]
                    )
                    nc.gpsimd.dma_start(
                        out=mfq[:, HG + h0 : HG + h0 + n, :],
                        in_=cc3_out[j][1],
                    )
            return mfq

        def wo_col_gen(qc, col, mfq):
            w_ps = fill_ps.tile([128, QC], F32, tag="fill")
            for hv in range(H):
                nc.tensor.matmul(
                    w_ps,
                    lhsT=wo_sb[:, hv, col * 128 : (col + 1) * 128],
                    rhs=mfq[:, hv, :],
                    start=(hv == 0),
                    stop=(hv == H - 1),
                )
                yield QC
            # drain on DVE and ship outT on the SYNC ring: the scalar
            # engine queue is the exp pipeline - its DMA_DIRECT2D slots
            # (~0.7us each) would add latency to every exp.
            ob = mfq_pool_box["ob"].tile([128, QC], F32, tag="ob", name="ob")
            nc.vector.tensor_copy(out=ob, in_=w_ps)
            # wo3's drains alternate rings: the ACT queue is exp-free by
            # then and splitting halves the end-of-kernel DMA drain.
            eng = nc.scalar if (qc == 3 and col % 2 == 1) else nc.sync
            eng.dma_start(
                out=outT[
                    col * 128 : (col + 1) * 128,
                    qc * QC : (qc + 1) * QC,
                ],
                in_=ob,
            )

        progress = {"q3": False}

        def filler_gen():
            # -- deferred Q2 heads 4-7: their xs stream is still
            # SBUF-resident, so these fills have no landing latency --
            for h in range(4, HG):
                ps = fill_ps.tile([128, QC], F32, tag="fill")
                for dci in range(NDC):
                    nc.tensor.matmul(
                        ps,
                        lhsT=wq_sb[:, dci, h * 128 : (h + 1) * 128],
                        rhs=xs_q2[:, dci, :],
                        start=(dci == 0),
                        stop=(dci == NDC - 1),
                    )
                    yield QC
                nc.vector.tensor_copy(
                    out=q_all[:, h, 2 * QC : 3 * QC], in_=ps
                )
            # -- deferred Q3 projection, in half-chunks of 256 --
            for half in (0, 1):
                xs = xq_tiles[half]
                for h in range(HG):
                    ps = fill_ps.tile([128, QC], F32, tag="fill")
                    for dci in range(NDC):
                        nc.tensor.matmul(
                            ps[:, : QC // 2],
                            lhsT=wq_sb[:, dci, h * 128 : (h + 1) * 128],
                            rhs=xs[:, dci, :],
                            start=(dci == 0),
                            stop=(dci == NDC - 1),
                        )
                        yield QC // 2
                    nc.vector.tensor_copy(
                        out=q_all[
                            :, h,
                            3 * QC + half * 256 : 3 * QC + (half + 1) * 256,
                        ],
                        in_=ps[:, : QC // 2],
                    )
            progress["q3"] = True
            # -- transition: free the x/wq SBUF, open the mfq + ob pools --
            stack_X.close()
            stack_A.close()
            mfq_pool_box["pool"] = stack_M.enter_context(
                tc.tile_pool(name="mfq", bufs=2)
            )
            mfq_pool_box["ob"] = stack_M.enter_context(
                tc.tile_pool(name="ob", bufs=2)
            )
            # -- wo chunks in gather order; the wo0/1/2 columns left
            # after att3 are the runway that covers the last chunk-3
            # pair exchanges --
            for qc in range(NQC):
                mfq = wo_load(qc)
                for col in range(NCOL):
                    yield from wo_col_gen(qc, col, mfq)

        filler = filler_gen()
        fstate = {"done": False}

        def fill(rows):
            while rows > 0 and not fstate["done"]:
                r = next(filler, None)
                if r is None:
                    fstate["done"] = True
                    return
                rows -= r

        def flush_q3():
            while not progress["q3"] and not fstate["done"]:
                fill(QC)

        def att_head(qc, h):
            filling = True
            # att0-att2 showed 82-94% PE feed at 1-block fills, so fill
            # harder there; att3 keeps a lighter rate so the wo tail
            # retains enough runway to cover the last per-head gathers.
            if qc == 0:
                f_full, f_thin = 768, 1024
            elif qc < 3:
                f_full, f_thin = QC, 768
            else:
                f_full, f_thin = 256, QC
            nkb = 4 * (qc + 1)
            ngrp = (nkb + 3) // 4
            # Process the diagonal-crossing k-group FIRST: its thin
            # (128..512-wide) s->exp->o chains then overlap the dense
            # full-width blocks instead of bunching at the head's end.
            # PSUM accumulation is order-independent; the group-first
            # block always has j0=0 either way.
            grp_order = [ngrp - 1] + list(range(ngrp - 1))
            kb_order = [4 * g + j for g in grp_order for j in range(4)
                        if 4 * g + j < nkb]
            o_ps = opsum.tile([128, QC], F32, tag="opsum")
            r_ps = rpsum.tile([128, QC], F32, tag="rpsum")
            racc = None
            for kbi, kb in enumerate(kb_order):
                j0 = kb_start(qc, kb)  # first live q col in chunk
                s_ps = spsum.tile([128, QC], F32, tag="spsum")
                nc.tensor.matmul(
                    s_ps[:, j0:],
                    lhsT=k_all[:, h, kb * 128 : (kb + 1) * 128],
                    rhs=q_all[:, h, qc * QC + j0 : (qc + 1) * QC],
                    start=True,
                    stop=True,
                )
                # Filler between the s matmul and its dependent o matmul
                # absorbs the exp round-trip latency; thin diagonal
                # blocks leave a bigger bubble, so fill more.
                if filling:
                    fill(f_full if j0 == 0 else f_thin)
                pt = pt_pool.tile([128, QC], BF16, tag="pt")
                nc.scalar.activation(
                    out=pt[:, j0:],
                    in_=s_ps[:, j0:],
                    func=mybir.ActivationFunctionType.Exp,
                    scale=float(SCALE),
                )
                if j0 < QC and 0 <= kb - 4 * qc:
                    # mask the diagonal-crossing 128 columns
                    nc.vector.tensor_mul(
                        out=pt[:, j0 : j0 + 128],
                        in0=pt[:, j0 : j0 + 128],
                        in1=tri_sb,
                    )
                nc.tensor.matmul(
                    o_ps[:, j0:],
                    lhsT=v_all[:, h, kb, :],
                    rhs=pt[:, j0:],
                    start=(kbi == 0),
                    stop=(kbi == nkb - 1),
                )
                # Denominators: sum ALL exp-blocks of the chunk on DVE
                # (bf16), then ONE full-width ones-matmul per head-chunk.
                # The first block in kb_order always has j0=0, so racc is
                # fully initialized.
                if kbi == 0:
                    racc = racc_pool.tile([128, QC], BF16, tag="racc")
                    nc.vector.tensor_copy(out=racc, in_=pt)
                else:
                    nc.vector.tensor_add(
                        out=racc[:, j0:], in0=racc[:, j0:], in1=pt[:, j0:]
                    )
                if kbi == nkb - 1:
                    nc.tensor.matmul(
                        r_ps, lhsT=ones_sb, rhs=racc, start=True, stop=True,
                    )
            # 1/r on DVE: single-instruction NR-seeded approximation
            # (~51 ULP) - the exact reciprocal held the rpsum bank
            # hostage and stalled the PE.
            rinv = rinv_pool.tile([128, QC], F32, tag="rinv")
            nc.vector.reciprocal_approx_fast(out=rinv, in_=r_ps)
            msb = mstage.tile([128, QC], BF16, tag="mstage")
            nc.vector.tensor_mul(out=msb, in0=o_ps, in1=rinv)
            # Exchange granularity is chosen so the CC stream (which
            # serializes ops at ~25us/MB + ~3us fixed) finishes the
            # last chunk-3 piece right behind att3: chunks 0/1 whole
            # (latency hidden), chunk 2 in halves, chunk 3 in pairs.
            if qc < 2:
                nc.sync.dma_start(out=cc_in[qc][:, h, :], in_=msb)
                if h == HG - 1:
                    nc.gpsimd.collective_compute(
                        "AllGather",
                        mybir.AluOpType.bypass,
                        ins=[cc_in[qc][:]],
                        outs=[cc_out[qc][:]],
                        replica_groups=pair_groups,
                    )
            elif qc == 2:
                nc.sync.dma_start(out=cc2_in[h // 4][:, h % 4, :], in_=msb)
                if h % 4 == 3:
                    nc.gpsimd.collective_compute(
                        "AllGather",
                        mybir.AluOpType.bypass,
                        ins=[cc2_in[h // 4][:]],
                        outs=[cc2_out[h // 4][:]],
                        replica_groups=pair_groups,
                    )
            else:
                j = min(h // 2, 2) if h < 6 else h - 3
                h0, n = cc3_parts[j]
                nc.sync.dma_start(out=cc3_in[j][:, h - h0, :], in_=msb)
                if h == h0 + n - 1:
                    nc.gpsimd.collective_compute(
                        "AllGather",
                        mybir.AluOpType.bypass,
                        ins=[cc3_in[j][:]],
                        outs=[cc3_out[j][:]],
                        replica_groups=pair_groups,
                    )

        for qc in range(NQC):
            if qc == 3:
                flush_q3()
            for h in range(HG):
                att_head(qc, h)
        # drain whatever filler remains (dense wo tail)
        while not fstate["done"]:
            fill(QC * 16)
        stack_M.close()

    nc.compile()
    return nc


def build_kernel_legacy(causal: bool):
    """Baseline schedule (kept for the non-causal mask fallback)."""
    nc = bacc.Bacc("TRN2", num_devices=N_CORES)

    qT = nc.declare_dram_parameter("qT", [D, T], BF16, isOutput=False)
    kT = nc.declare_dram_parameter("kT", [D, T], BF16, isOutput=False)
    vT = nc.declare_dram_parameter("vT", [D, T], BF16, isOutput=False)
    wq = nc.declare_dram_parameter("wq", [D, HG * DK], BF16, isOutput=False)
    wk = nc.declare_dram_parameter("wk", [D, HG * DK], BF16, isOutput=False)
    wv = nc.declare_dram_parameter("wv", [D, HG * DV], BF16, isOutput=False)
    wo = nc.declare_dram_parameter("wo", [H * DV, D // 2], BF16, isOutput=False)
    tri = nc.declare_dram_parameter("tri", [128, 128], BF16, isOutput=False)
    if not causal:
        maskT = nc.declare_dram_parameter("maskT", [T, T], BF16, isOutput=False)
    outT = nc.declare_dram_parameter("outT", [D // 2, T], F32, isOutput=True)

    cc_in = {qc: nc.dram_tensor(f"cc_in_{qc}", [128, HG, QC], BF16)
             for qc in range(NQC)}
    cc_out = {qc: nc.dram_tensor(f"cc_out_{qc}", [2, 128, HG, QC], BF16)
              for qc in range(NQC)}
    cc_warm_in = nc.dram_tensor("cc_warm_in", [128, 8], BF16)
    cc_warm_out = nc.dram_tensor("cc_warm_out", [2, 128, 8], BF16)
    pair_groups = [[0, 1], [2, 3], [4, 5], [6, 7]]

    ATT_ORDER = (3, 0, 2, 1)

    def kb_start(qc, kb):
        if not causal:
            return 0
        return min(max((kb - 4 * qc) * 128, 0), QC)

    with tile.TileContext(nc) as tc, ExitStack() as top:
        ent = top.enter_context
        consts = ent(tc.tile_pool(name="consts", bufs=1))
        res_pool = ent(tc.tile_pool(name="res", bufs=1))
        proj_stack = ExitStack()
        w_pool = proj_stack.enter_context(tc.tile_pool(name="w", bufs=2))
        x_pool = proj_stack.enter_context(tc.tile_pool(name="xs", bufs=2))

        ones_sb = consts.tile([128, 128], BF16)
        nc.vector.memset(ones_sb, 1.0)
        tri_sb = consts.tile([128, 128], BF16)
        nc.sync.dma_start(out=tri_sb, in_=tri[:])

        nc.gpsimd.collective_compute(
            "AllGather",
            mybir.AluOpType.bypass,
            ins=[cc_warm_in[:]],
            outs=[cc_warm_out[:]],
            replica_groups=pair_groups,
        )

        q_all = res_pool.tile([128, HG, T], BF16, name="q_all")
        k_all = res_pool.tile([128, HG, T], BF16, name="k_all")
        v_all = res_pool.tile([128, HG, NKB, DV], BF16, name="v_all")

        def weight_tile(pool):
            return pool.tile([128, NDC, HG * 128], BF16, tag="w", name="wtile")

        def weight_slice(w_sb, w_ext, dci):
            nc.sync.dma_start(
                out=w_sb[:, dci, :], in_=w_ext[dci * 128 : (dci + 1) * 128, :]
            )

        wv_sb = weight_tile(w_pool)
        for c in range(4):
            nc.sync.dma_start(
                out=wv_sb[:, 4 * c : 4 * (c + 1), :],
                in_=wv[4 * c * 128 : 4 * (c + 1) * 128, :].rearrange(
                    "(o p) f -> p o f", p=128
                ),
            )
        wk_sb = weight_tile(w_pool)
        wq_sb = None

        def x_stream(src, qc):
            xs = x_pool.tile([128, NDC, QC], BF16, tag="xs")
            nc.scalar.dma_start(
                out=xs,
                in_=src[:, qc * QC : (qc + 1) * QC].rearrange(
                    "(o p) f -> p o f", p=128
                ),
            )
            return xs

        with (
            tc.tile_pool(name="warmps", bufs=1, space="PSUM") as warmps,
            tc.tile_pool(name="ppsum", bufs=4, space="PSUM") as ppsum,
        ):
            wps = warmps.tile([128, 128], F32)
            for i in range(290):
                nc.tensor.matmul(
                    wps, lhsT=ones_sb, rhs=ones_sb,
                    start=(i == 0), stop=(i == 289),
                )

            for qv in range(NQC):
                xv = x_stream(vT, qv)
                for dci in (range(4) if qv == 0 else []):
                    weight_slice(wk_sb, wk, dci)
                for kbs in range(4):
                    kb = 4 * qv + kbs
                    if kbs == 3 and qv < NQC - 1:
                        for dci in range(4 * (qv + 1), 4 * (qv + 2)):
                            weight_slice(wk_sb, wk, dci)
                    for nn in range(2):
                        ps = ppsum.tile([128, 512], F32, tag="pp")
                        for dci in range(NDC):
                            nc.tensor.matmul(
                                ps,
                                lhsT=xv[:, dci, kbs * 128 : (kbs + 1) * 128],
                                rhs=wv_sb[:, dci, nn * 512 : (nn + 1) * 512],
                                start=(dci == 0),
                                stop=(dci == NDC - 1),
                            )
                        nc.vector.tensor_copy(
                            out=v_all[:, nn * 4 : (nn + 1) * 4, kb, :], in_=ps
                        )

            wq_sb = weight_tile(w_pool)
            for qc in range(NQC):
                xs = x_stream(kT, qc)
                for dci in range(4 * qc, 4 * qc + 4):
                    weight_slice(wq_sb, wq, dci)
                for h in range(HG):
                    ps = ppsum.tile([128, QC], F32, tag="pp")
                    for dci in range(NDC):
                        nc.tensor.matmul(
                            ps,
                            lhsT=wk_sb[:, dci, h * 128 : (h + 1) * 128],
                            rhs=xs[:, dci, :],
                            start=(dci == 0),
                            stop=(dci == NDC - 1),
                        )
                    nc.vector.tensor_copy(
                        out=k_all[:, h, qc * QC : (qc + 1) * QC], in_=ps
                    )

            for qc in ATT_ORDER:
                xs = x_stream(qT, qc)
                for h in range(HG):
                    ps = ppsum.tile([128, QC], F32, tag="pp")
                    for dci in range(NDC):
                        nc.tensor.matmul(
                            ps,
                            lhsT=wq_sb[:, dci, h * 128 : (h + 1) * 128],
                            rhs=xs[:, dci, :],
                            start=(dci == 0),
                            stop=(dci == NDC - 1),
                        )
                    nc.vector.tensor_copy(
                        out=q_all[:, h, qc * QC : (qc + 1) * QC], in_=ps
                    )

        proj_stack.close()

        wo_pool = ent(tc.tile_pool(name="wop", bufs=1))
        wo_sb = wo_pool.tile([128, NDC, D // 2], BF16, name="wo_sb")
        for dci in range(NDC):
            weight_slice(wo_sb, wo, dci)
        pt_pool = ent(tc.tile_pool(name="pt", bufs=10 if causal else 6))
        racc_pool = ent(tc.tile_pool(name="racc", bufs=4 if causal else 3))
        mstage = ent(tc.tile_pool(name="mstage", bufs=4 if causal else 3))
        rinv_pool = ent(tc.tile_pool(name="rinv", bufs=3 if causal else 2))
        mfq_pool = ent(tc.tile_pool(name="mfq", bufs=2 if causal else 1))
        ob_pool = ent(tc.tile_pool(name="ob", bufs=4))
        gm_pool = ent(tc.tile_pool(name="gm", bufs=2)) if not causal else None
        spsum = ent(tc.tile_pool(name="spsum", bufs=2, space="PSUM"))
        opsum = ent(tc.tile_pool(name="opsum", bufs=2, space="PSUM"))
        rpsum = ent(tc.tile_pool(name="rpsum", bufs=2, space="PSUM"))
        wpsum = ent(tc.tile_pool(name="wpsum", bufs=2, space="PSUM"))

        def load_gm(qc):
            if causal:
                return None
            gm = gm_pool.tile([128, NKB, QC], BF16, tag="gm")
            nc.scalar.dma_start(
                out=gm,
                in_=maskT[:, qc * QC : (qc + 1) * QC].rearrange(
                    "(o p) f -> p o f", p=128
                ),
            )
            return gm

        def att_head(qc, h, gm):
            nkb = 4 * (qc + 1) if causal else NKB
            ngrp = (nkb + 3) // 4
            grp_order = ([ngrp - 1] + list(range(ngrp - 1))) if causal else \
                list(range(ngrp))
            kb_order = [4 * g + j for g in grp_order for j in range(4)
                        if 4 * g + j < nkb]
            o_ps = opsum.tile([128, QC], F32, tag="opsum")
            r_ps = rpsum.tile([128, QC], F32, tag="rpsum")
            racc = None
            for kbi, kb in enumerate(kb_order):
                j0 = kb_start(qc, kb)
                s_ps = spsum.tile([128, QC], F32, tag="spsum")
                nc.tensor.matmul(
                    s_ps[:, j0:],
                    lhsT=k_all[:, h, kb * 128 : (kb + 1) * 128],
                    rhs=q_all[:, h, qc * QC + j0 : (qc + 1) * QC],
                    start=True,
                    stop=True,
                )
                pt = pt_pool.tile([128, QC], BF16, tag="pt")
                nc.scalar.activation(
                    out=pt[:, j0:],
                    in_=s_ps[:, j0:],
                    func=mybir.ActivationFunctionType.Exp,
                    scale=float(SCALE),
                )
                if causal:
                    if j0 < QC and kb - 4 * qc >= 0:
                        nc.vector.tensor_mul(
                            out=pt[:, j0 : j0 + 128],
                            in0=pt[:, j0 : j0 + 128],
                            in1=tri_sb,
                        )
                else:
                    nc.vector.tensor_mul(out=pt, in0=pt, in1=gm[:, kb, :])
                nc.tensor.matmul(
                    o_ps[:, j0:],
                    lhsT=v_all[:, h, kb, :],
                    rhs=pt[:, j0:],
                    start=(kbi == 0),
                    stop=(kbi == nkb - 1),
                )
                if kbi % 4 == 0:
                    racc = racc_pool.tile([128, QC], BF16, tag="racc")
                    nc.vector.tensor_copy(out=racc, in_=pt)
                else:
                    nc.vector.tensor_add(
                        out=racc[:, j0:], in0=racc[:, j0:], in1=pt[:, j0:]
                    )
                if kbi % 4 == 3 or kbi == nkb - 1:
                    nc.tensor.matmul(
                        r_ps,
                        lhsT=ones_sb,
                        rhs=racc,
                        start=(kbi // 4 == 0),
                        stop=(kbi // 4 == ngrp - 1),
                    )
            rinv = rinv_pool.tile([128, QC], F32, tag="rinv")
            nc.vector.reciprocal_approx_fast(out=rinv, in_=r_ps)
            msb = mstage.tile([128, QC], BF16, tag="mstage")
            nc.vector.tensor_mul(out=msb, in0=o_ps, in1=rinv)
            nc.sync.dma_start(out=cc_in[qc][:, h, :], in_=msb)
            if h == HG - 1:
                nc.gpsimd.collective_compute(
                    "AllGather",
                    mybir.AluOpType.bypass,
                    ins=[cc_in[qc][:]],
                    outs=[cc_out[qc][:]],
                    replica_groups=pair_groups,
                )

        def wo_load(qc):
            mfq = mfq_pool.tile([128, H, QC], BF16, tag="mfq")
            nc.sync.dma_start(out=mfq[:, :HG, :], in_=cc_out[qc][0])
            nc.sync.dma_start(out=mfq[:, HG:, :], in_=cc_out[qc][1])
            return mfq

        def wo_col(qc, col, mfq):
            w_ps = wpsum.tile([128, QC], F32, tag="wpsum")
            for hv in range(H):
                nc.tensor.matmul(
                    w_ps,
                    lhsT=wo_sb[:, hv, col * 128 : (col + 1) * 128],
                    rhs=mfq[:, hv, :],
                    start=(hv == 0),
                    stop=(hv == H - 1),
                )
            ob = ob_pool.tile([128, QC], F32, tag="ob")
            nc.scalar.activation(
                out=ob, in_=w_ps, func=mybir.ActivationFunctionType.Copy
            )
            nc.scalar.dma_start(
                out=outT[
                    col * 128 : (col + 1) * 128,
                    qc * QC : (qc + 1) * QC,
                ],
                in_=ob,
            )

        gm3 = load_gm(3)
        for h in range(HG):
            att_head(3, h, gm3)
        gm0 = load_gm(0)
        for h in range(HG):
            att_head(0, h, gm0)
        mfq3 = wo_load(3)
        gm2 = load_gm(2)
        for h in range(HG):
            att_head(2, h, gm2)
        mfq0 = wo_load(0)
        gm1 = load_gm(1)
        for h in range(HG):
            att_head(1, h, gm1)
            wo_col(3, h, mfq3)
        mfq2 = wo_load(2)
        for col in range(NCOL):
            wo_col(0, col, mfq0)
        mfq1 = wo_load(1)
        for col in range(NCOL):
            wo_col(2, col, mfq2)
        for col in range(NCOL):
            wo_col(1, col, mfq1)

    nc.compile()
    return nc


def kernel(q, k, v, mask, Wq, Wk, Wv, Wo):
    q = np.asarray(q)
    k = np.asarray(k)
    v = np.asarray(v)
    mask = np.asarray(mask)
    causal = bool(np.array_equal(mask, np.tril(np.ones((T, T), dtype=bool))))

    if causal not in _KERNEL_CACHE:
        _KERNEL_CACHE[causal] = (
            build_kernel_causal() if causal else build_kernel_legacy(False)
        )
    nc = _KERNEL_CACHE[causal]

    bf = ml_dtypes.bfloat16
    Wq_b = np.asarray(Wq).astype(bf)
    Wk_b = np.asarray(Wk).astype(bf)
    Wv_b = np.asarray(Wv).astype(bf)
    Wo_b = np.asarray(Wo).astype(bf)
    i = np.arange(128)
    tri_np = (i[None, :] >= i[:, None]).astype(bf)  # tri[k, j] = j >= k
    maskT_np = None if causal else np.ascontiguousarray(mask.T).astype(bf)

    in_maps = []
    for c in range(N_CORES):
        b, g = c // 2, c % 2
        m = {
            "qT": np.ascontiguousarray(q[b].T).astype(bf),
            "kT": np.ascontiguousarray(k[b].T).astype(bf),
            "vT": np.ascontiguousarray(v[b].T).astype(bf),
            "wq": np.ascontiguousarray(Wq_b[:, g * 1024 : (g + 1) * 1024]),
            "wk": np.ascontiguousarray(Wk_b[:, g * 1024 : (g + 1) * 1024]),
            "wv": np.ascontiguousarray(Wv_b[:, g * 1024 : (g + 1) * 1024]),
            "wo": np.ascontiguousarray(Wo_b[:, g * 1024 : (g + 1) * 1024]),
            "tri": tri_np,
        }
        if not causal:
            m["maskT"] = maskT_np
        in_maps.append(m)

    trace = bool(os.environ.get("BASS_KERNEL_TRACE")) and (
        "antenv.axon_hooks" in sys.modules
    )
    res = run_bass_kernel_spmd(nc, in_maps, list(range(N_CORES)), trace=trace)
    if trace and res.exec_time_ns is not None:
        print(f"HW exec time: {res.exec_time_ns} ns")
        kernel.last_exec_time_ns = res.exec_time_ns
        kernel.last_results = res

    out = np.empty((B, T, D), dtype=np.float32)
    for b in range(B):
        top = res.results[2 * b]["outT"]        # cols 0..1023, [1024, 2048]
        bot = res.results[2 * b + 1]["outT"]    # cols 1024..2047
        out[b] = np.concatenate([top, bot], axis=0).T
    return out
